# revision 9
# baseline (speedup 1.0000x reference)
"""ChildSumTreeLSTM with relation transforms on 8 Trainium2 NeuronCores.

Layout: everything transposed (features on SBUF partitions, tree nodes on the
free dim), node columns in topological-wave order (= heap order for the
reference tree). Per cooperative wave of parents:
  hsum (DVE strided reduce over child cols) -> per-slot masked copies of hsum
  (column masks are input data; zero cols whose rel != slot rel) -> rel-sharded
  PE streams of SBUF-RESIDENT fp8 W blocks, all slots accumulated into one
  PSUM group (masked inputs make contributions disjoint across cores) ->
  dense bf16 AllReduce gives every core the full ch_sum -> column-sharded
  iou/f gates (each core owns one 128-feature slice) -> small AllGather of the
  new h columns.
The last waves (tiny: <= TAILN nodes each) run fully replicated on every core
with full-width fp8 gate weights and replicated relation matrices, so they
need no collectives; the host reads those columns from a separate full-width
output. Full-width c / xi / xf for the tail ride piggyback on the publish
AllGathers.
The W relation matrices are held resident in SBUF in fp8 (scaled by 16 to
avoid fp8 subnormals; the 1/16 is folded into the iouh weights), so the wave
loop performs no weight DMA. Bulk preloads go on the Activation DMA queue;
latency-critical transfers (collective bounces etc.) use the SP queue.
All per-core differences are input data (weight shards, masks, bias slices),
so one Bass program runs SPMD on all 8 cores.
"""

import sys

sys.path.insert(0, "/opt/trn_rl_repo")

import numpy as np
import ml_dtypes

import concourse.bass as bass
import concourse.mybir as mybir
import concourse.tile as tile
from concourse.bass_utils import run_bass_kernel_spmd
from concourse.vector_clock import ScopedClock, VectorClock

BF16 = mybir.dt.bfloat16
F32 = mybir.dt.float32
FP8 = mybir.dt.float8e4
NCORES = 8
P = 128
WSCALE = 16.0
TAILN = 3   # waves with <= this many parents run replicated (no collectives)

# This walrus build rejects >1 sem wait per instruction at the Tile exit
# drain; split the aggregated drain into one drain per proc.
def _split_drain_and_barrier(self, tick_clock, wait_clock):
    gc = tick_clock.global_clock
    n = len(gc)
    nonzero = [i for i in range(n) if gc[i] > 0]
    for j in nonzero:
        vec = VectorClock([gc[i] if i == j else 0 for i in range(n)])
        d = self.nc.sync.drain()
        wait_clock.add_sem_waits(d.ins, ScopedClock({None: vec}))
    if not nonzero:
        d = self.nc.sync.drain()
        wait_clock.add_sem_waits(d.ins, ScopedClock({None: gc.copy()}))
    self.nc.all_engine_barrier()
    assert self.sems is not None
    popped = self.nc._tile_sem_poison_stack.pop()
    assert popped is self._sem_poison
    self.nc.clear_and_free_semaphores(list(self.sems.allocated().values()))
    self.nc.all_engine_barrier()


tile.TileContext._drain_and_barrier = _split_drain_and_barrier


def _split_multi_waits(nc, limit=1):
    """Walrus here allows only one sem wait per instruction; hoist extras
    onto same-engine NOPs inserted right before the instruction."""
    for bb in nc.main_func.blocks:
        new_list = []
        for ins in bb.instructions:
            si = getattr(ins, "sync_info", None)
            if si is not None and si.on_wait and len(si.on_wait) > limit:
                waits = list(si.on_wait)
                for w in waits[:-limit]:
                    nop = mybir.InstNoOp(
                        name=nc.get_next_instruction_name(),
                        sync_info=mybir.SyncInfo(on_wait=[w], on_update=[]),
                        bass_nofuse=True,
                        engine=ins.engine,
                    )
                    nc.register_instruction(nop, overwrite=True)
                    new_list.append(nop)
                si.on_wait = waits[-limit:]
            new_list.append(ins)
        bb.instructions[:] = new_list


def _bf16(a):
    return np.ascontiguousarray(a.astype(ml_dtypes.bfloat16))


def _fp8(a):
    return np.ascontiguousarray(a.astype(ml_dtypes.float8_e4m3))


def _blocksT(mat):
    """[M, K] -> [M/128 * K/128, 128, 128] of transposed blocks, k-major order
    grouped as [m, k] -> index m*KC + k, each block = mat[mb, kb].T (lhsT)."""
    M, K = mat.shape
    MC, KC = M // P, K // P
    out = np.empty((MC * KC, P, P), mat.dtype)
    for m in range(MC):
        for k in range(KC):
            out[m * KC + k] = mat[m * P:(m + 1) * P, k * P:(k + 1) * P].T
    return out


def _plan(child_idx, rel_ids, Wrel):
    """Host-side planning: waves, column order, rel->core assignment, slots."""
    N, K = child_idx.shape
    eff_children = []
    wave = np.zeros(N, np.int32)
    for i in range(N):
        cs = [int(c) for c in child_idx[i] if 0 <= c < i]
        eff_children.append(cs)
        wave[i] = 1 + max((wave[c] for c in cs), default=-1)
    nwaves = int(wave.max()) + 1
    # column order: by (wave, descending node) -> for the reference heap tree
    # this is exactly heap order, keeping children of consecutive parents
    # contiguous.
    order = sorted(range(N), key=lambda i: (wave[i], -i))
    col_of = np.empty(N, np.int64)
    for j, node in enumerate(order):
        col_of[node] = j
    waves = []  # list of (p0, p1) col ranges
    j = 0
    for w in range(nwaves):
        cnt = int((wave == w).sum())
        waves.append((j, j + cnt))
        j += cnt

    ident = set()
    eye = np.eye(Wrel.shape[1], dtype=Wrel.dtype)
    for r in set(int(rel_ids[i]) for i in range(N)):
        if np.array_equal(Wrel[r], eye):
            ident.add(r)

    # first tail wave: every wave from tail_w0 on is tiny and runs replicated
    tail_w0 = nwaves
    while tail_w0 - 1 >= 1 and waves[tail_w0 - 1][1] - waves[tail_w0 - 1][0] <= TAILN:
        tail_w0 -= 1

    # per cooperative wave (1..tail_w0-1): rels present; identity-only waves
    # skip the relation matmul entirely
    wave_rels = []
    for w in range(1, tail_w0):
        p0, p1 = waves[w]
        rels_all = set(int(rel_ids[order[j]]) for j in range(p0, p1))
        if rels_all <= ident:
            wave_rels.append([])
        else:
            wave_rels.append(sorted(rels_all))

    # static rel->core assignment, greedy balance on total appearances
    from collections import defaultdict
    count = defaultdict(int)
    for rels in wave_rels:
        for r in rels:
            count[r] += 1
    nw = len(wave_rels)
    loadw = [[0] * nw for _ in range(NCORES)]
    assign = {}
    for r in sorted(count, key=lambda r: -count[r]):
        pres = [wi for wi in range(nw) if r in wave_rels[wi]]
        best, bkey = 0, None
        for c in range(NCORES):
            key = (sum(loadw[c][wi] for wi in pres), sum(loadw[c]))
            if bkey is None or key < bkey:
                best, bkey = c, key
        assign[r] = best
        for wi in pres:
            loadw[best][wi] += 1

    wave_slots = []
    for rels in wave_rels:
        per_core = [[r for r in rels if assign[r] == c] for c in range(NCORES)]
        n_s = max((len(x) for x in per_core), default=0)
        wave_slots.append((n_s, per_core))

    # tail waves: replicated slots, one per non-identity rel per wave
    tail_slots = []
    for w in range(tail_w0, nwaves):
        p0, p1 = waves[w]
        rels_all = set(int(rel_ids[order[j]]) for j in range(p0, p1))
        if rels_all <= ident:
            tail_slots.append([])
        else:
            tail_slots.append(sorted(rels_all))

    # cols in cooperative waves whose full-width c a tail wave needs
    tail_child_cols = sorted(set(
        int(col_of[c]) for w in range(tail_w0, nwaves)
        for j in range(waves[w][0], waves[w][1])
        for c in eff_children[order[j]]
        if wave[c] < tail_w0))

    return dict(order=order, col_of=col_of, waves=waves, wave=wave,
                eff_children=eff_children, ident=ident,
                wave_slots=wave_slots, nwaves=nwaves, tail_w0=tail_w0,
                tail_slots=tail_slots, tail_child_cols=tail_child_cols)


def _build(inputs):
    x = np.asarray(inputs["x"], np.float32)
    Wrel = np.asarray(inputs["Wrel"], np.float32)
    ioux_w = np.asarray(inputs["ioux_w"], np.float32)
    ioux_b = np.asarray(inputs["ioux_b"], np.float32)
    iouh_w = np.asarray(inputs["iouh_w"], np.float32)
    iouh_b = np.asarray(inputs["iouh_b"], np.float32)
    fx_w = np.asarray(inputs["fx_w"], np.float32)
    fx_b = np.asarray(inputs["fx_b"], np.float32)
    fh_w = np.asarray(inputs["fh_w"], np.float32)
    fh_b = np.asarray(inputs["fh_b"], np.float32)
    child_idx = np.asarray(inputs["child_idx"], np.int32)
    rel_ids = np.asarray(inputs["rel_ids"], np.int32)

    N, IN_DIM = x.shape
    MEM = fh_w.shape[0]
    KC = MEM // P           # 8 feature chunks
    KX = IN_DIM // P        # 8 input chunks
    K = child_idx.shape[1]  # max children (4)
    NPAD = N + K + 4
    assert KC == NCORES  # publish-AG rank axis doubles as the row-chunk axis

    plan = _plan(child_idx, rel_ids, Wrel)
    order, col_of, waves = plan["order"], plan["col_of"], plan["waves"]
    eff_children = plan["eff_children"]
    wave_slots, nwaves = plan["wave_slots"], plan["nwaves"]
    tail_w0, tail_slots = plan["tail_w0"], plan["tail_slots"]
    tail_child_cols = plan["tail_child_cols"]
    wave_arr = plan["wave"]

    # Child gather plan: per internal wave, the flattened (parent-major)
    # child column sequence, decomposed into maximal +1-contiguous runs.
    ZCOL = N
    child_col = np.full((N, K), ZCOL, np.int64)
    for i in range(N):
        for kk, c in enumerate(eff_children[i]):
            child_col[i, kk] = col_of[c]
    wave_runs = []
    for w in range(1, nwaves):
        p0, p1 = waves[w]
        seq = []
        for j in range(p0, p1):
            seq.extend(child_col[order[j]])
        runs = []
        i0 = 0
        while i0 < len(seq):
            i1 = i0 + 1
            while i1 < len(seq) and seq[i1] == seq[i1 - 1] + 1:
                i1 += 1
            runs.append((i0, int(seq[i0]), i1 - i0))
            i0 = i1
        wave_runs.append(runs)

    # tail bookkeeping
    TC = len(tail_child_cols)
    tail_nodes = [order[j] for w in range(tail_w0, nwaves)
                  for j in range(waves[w][0], waves[w][1])]
    TN = len(tail_nodes)
    cful_idx = {c: i for i, c in enumerate(tail_child_cols)}
    for i, nd in enumerate(tail_nodes):
        cful_idx[int(col_of[nd])] = TC + i
    pig_cols = []  # per wave 0..tail_w0-1: list of (col_in_wave, cful_pos)
    for w in range(0, tail_w0):
        p0, p1 = waves[w]
        pig_cols.append([(j - p0, cful_idx[j]) for j in range(p0, p1)
                         if j in cful_idx])

    # ---- per-core host data -------------------------------------------------
    xT = np.ascontiguousarray(x[order].T)
    xT_b = np.zeros((KX, P, N), ml_dtypes.bfloat16)
    for k in range(KX):
        xT_b[k] = _bf16(xT[k * P:(k + 1) * P])

    S_coop = sum(ns for ns, _ in wave_slots)
    S_tail = sum(len(rl) for rl in tail_slots)
    ST = max(S_coop + S_tail, 1)
    MC = MEM // P
    NMAX = max((waves[w][1] - waves[w][0]) for w in range(1, nwaves)) if nwaves > 1 else 1
    NBIG = max(p1 - p0 for p0, p1 in waves)
    PSN = 128  # psum column pad so each m-chunk slice stays inside one bank
    assert NMAX <= PSN and K * NMAX <= 512

    wres = [np.zeros((ST, P, MC * KC, P), ml_dtypes.float8_e4m3)
            for _ in range(NCORES)]
    masks = [np.zeros((ST, NMAX), np.float32) for _ in range(NCORES)]
    soff = 0
    for wi, (ns, per_core) in enumerate(wave_slots):
        w = wi + 1
        p0, p1 = waves[w]
        n = p1 - p0
        for c in range(NCORES):
            for s, r in enumerate(per_core[c]):
                wres[c][soff + s] = _fp8(
                    (_blocksT(Wrel[r]) * WSCALE).transpose(1, 0, 2))
                for t in range(n):
                    if int(rel_ids[order[p0 + t]]) == r:
                        masks[c][soff + s, t] = 1.0
        soff += ns
    tail_soff = []
    for twi, rl in enumerate(tail_slots):
        w = tail_w0 + twi
        p0, p1 = waves[w]
        n = p1 - p0
        tail_soff.append(soff)
        for s, r in enumerate(rl):
            blk = _fp8((_blocksT(Wrel[r]) * WSCALE).transpose(1, 0, 2))
            for c in range(NCORES):
                wres[c][soff + s] = blk
                for t in range(n):
                    if int(rel_ids[order[p0 + t]]) == r:
                        masks[c][soff + s, t] = 1.0
        soff += len(rl)
    masks_b = [np.ascontiguousarray(np.broadcast_to(
        m[None], (P,) + m.shape).astype(ml_dtypes.bfloat16))
        for m in masks]

    iouxstat = [np.zeros((KX * 3, P, P), ml_dtypes.bfloat16) for _ in range(NCORES)]
    fxstat = [np.zeros((KX, P, P), ml_dtypes.bfloat16) for _ in range(NCORES)]
    fhstat = [np.zeros((KC, P, P), ml_dtypes.bfloat16) for _ in range(NCORES)]
    b_xi = [np.zeros((3, P), np.float32) for _ in range(NCORES)]
    b_iou = [np.zeros((3, P), np.float32) for _ in range(NCORES)]
    b_xf = [np.zeros((P,), np.float32) for _ in range(NCORES)]
    b_fh = [np.zeros((P,), np.float32) for _ in range(NCORES)]
    for c in range(NCORES):
        rows = slice(c * P, (c + 1) * P)
        for g in range(3):
            gr = slice(g * MEM + c * P, g * MEM + (c + 1) * P)
            b_xi[c][g] = ioux_b[gr]
            b_iou[c][g] = iouh_b[gr]
            for k in range(KX):
                iouxstat[c][k * 3 + g] = _bf16(
                    ioux_w[gr, k * P:(k + 1) * P].T)
        b_xf[c] = fx_b[rows]
        b_fh[c] = fh_b[rows]
        for k in range(KX):
            fxstat[c][k] = _bf16(fx_w[rows, k * P:(k + 1) * P].T)
        for k in range(KC):
            fhstat[c][k] = _bf16(fh_w[rows, k * P:(k + 1) * P].T)

    # full-width iouh in bf16, per-core block order: m-slot-major with the
    # core's own row-chunk in slot 0. Serves both the row-sharded coop iou
    # (slot 0, uniform) and the full-width replicated tail iou (all slots;
    # rows permuted on cores != 0, whose tail output is unused).
    # rhs carries WSCALE, compensated by 1/WSCALE at the gates.
    iouh_blk = _blocksT(iouh_w)  # [(g*KC+m)*KC+k, P, P]
    iouh_full = []
    for c in range(NCORES):
        sig = [c] + [m for m in range(KC) if m != c]
        buf = np.zeros((KC * 3 * KC, P, P), ml_dtypes.bfloat16)
        for ms in range(KC):
            for g in range(3):
                for k in range(KC):
                    buf[(ms * 3 + g) * KC + k] = _bf16(
                        iouh_blk[(g * KC + sig[ms]) * KC + k])
        iouh_full.append(buf)
    # full-width fh for intra-tail children only, fp8 (error impact ~nil),
    # identity row order (misaligned rows on cores != 0 -> unused garbage)
    fh_full = _fp8(_blocksT(fh_w * WSCALE))          # [KC*KC, P, P]

    # ---- build program ------------------------------------------------------
    nc = bass.Bass("TRN2", target_bir_lowering=False, debug=False,
                   num_devices=NCORES)
    d_ws = nc.dram_tensor("wres", list(wres[0].shape), FP8,
                          kind="ExternalInput")
    d_mask = nc.dram_tensor("masks", list(masks_b[0].shape), BF16,
                            kind="ExternalInput")
    d_xt = nc.dram_tensor("xt", [KX, P, N], BF16, kind="ExternalInput")
    d_iouxs = nc.dram_tensor("iouxstat", [KX * 3, P, P], BF16, kind="ExternalInput")
    d_fxs = nc.dram_tensor("fxstat", [KX, P, P], BF16, kind="ExternalInput")
    d_fhs = nc.dram_tensor("fhstat", [KC, P, P], BF16, kind="ExternalInput")
    d_iouhf = nc.dram_tensor("iouh_full", [KC * 3 * KC, P, P], BF16,
                             kind="ExternalInput")
    d_fhf = nc.dram_tensor("fh_full", [KC * KC, P, P], FP8,
                           kind="ExternalInput")
    d_bxi = nc.dram_tensor("b_xi", [3, P], F32, kind="ExternalInput")
    d_biou = nc.dram_tensor("b_iou", [3, P], F32, kind="ExternalInput")
    d_bxf = nc.dram_tensor("b_xf", [P], F32, kind="ExternalInput")
    d_bfh = nc.dram_tensor("b_fh", [P], F32, kind="ExternalInput")
    d_hout = nc.dram_tensor("hout", [P, N], F32, kind="ExternalOutput")
    d_tailh = nc.dram_tensor("tailh", [P, KC, max(TN, 1)], F32,
                             kind="ExternalOutput")

    wave_soff = []
    soff = 0
    for ns, _ in wave_slots:
        wave_soff.append(soff)
        soff += ns

    with tile.TileContext(nc, num_cores=NCORES) as tc:
        with (
            tc.tile_pool(name="const", bufs=1) as cpool,
            tc.tile_pool(name="state", bufs=1) as spool,
            tc.tile_pool(name="hmp", bufs=3) as hmp,
            tc.tile_pool(name="work", bufs=1) as wk,
            tc.tile_pool(name="psum", bufs=1, space="PSUM") as pp,
            tc.tile_pool(name="psg", bufs=2, space="PSUM") as pg,
            tc.tile_pool(name="dram", bufs=2, space="DRAM") as dp,
        ):
            # constants for the xi/xf precompute + leaf wave first (Act queue)
            xt = cpool.tile([P, KX, N], BF16)
            nc.scalar.dma_start(xt[:], d_xt.ap().rearrange("k p n -> p k n"))
            iouxs = cpool.tile([P, KX * 3, P], BF16)
            nc.scalar.dma_start(iouxs[:], d_iouxs.ap().rearrange("s p m -> p s m"))
            fxs = cpool.tile([P, KX, P], BF16)
            nc.scalar.dma_start(fxs[:], d_fxs.ap().rearrange("s p m -> p s m"))
            bxi = cpool.tile([P, 3], F32)
            nc.scalar.dma_start(bxi[:], d_bxi.ap().rearrange("g p -> p g"))
            biou = cpool.tile([P, 3], F32)
            nc.scalar.dma_start(biou[:], d_biou.ap().rearrange("g p -> p g"))
            bxf = cpool.tile([P, 1], F32)
            nc.scalar.dma_start(bxf[:], d_bxf.ap().rearrange("(p one) -> p one", one=1))
            bfh = cpool.tile([P, 1], F32)
            nc.scalar.dma_start(bfh[:], d_bfh.ap().rearrange("(p one) -> p one", one=1))

            # resident relation weights + masks, in wave order (Act queue)
            wres_sb = cpool.tile([P, ST * MC * KC, P], FP8)
            msk_sb = cpool.tile([P, ST, NMAX], BF16)
            fhs = cpool.tile([P, KC, P], BF16)
            iouhf = cpool.tile([P, KC * 3 * KC, P], BF16)
            fhf = cpool.tile([P, KC * KC, P], FP8)

            def load_slots(lo, hi):
                for s in range(lo, hi):
                    nc.scalar.dma_start(
                        wres_sb[:, s * MC * KC:(s + 1) * MC * KC, :],
                        d_ws.ap()[s])
                if hi > lo:
                    nc.scalar.dma_start(msk_sb[:, lo:hi, :],
                                        d_mask.ap()[:, lo:hi])

            for wi in range(1, tail_w0):
                ns = wave_slots[wi - 1][0]
                so = wave_soff[wi - 1]
                load_slots(so, so + ns)
                if wi == 1:
                    nc.scalar.dma_start(
                        iouhf[:, :3 * KC, :],
                        d_iouhf.ap()[:3 * KC].rearrange("s p m -> p s m"))
                    nc.scalar.dma_start(
                        fhs[:], d_fhs.ap().rearrange("s p m -> p s m"))
            # tail weights last (needed latest)
            load_slots(S_coop, S_coop + S_tail)
            nc.scalar.dma_start(
                iouhf[:, 3 * KC:, :],
                d_iouhf.ap()[3 * KC:].rearrange("s p m -> p s m"))
            nc.scalar.dma_start(fhf[:], d_fhf.ap().rearrange("s p m -> p s m"))

            # state
            h_bf = spool.tile([P, KC, NPAD], BF16)
            nc.vector.memset(h_bf[:], 0.0)
            c_sl = spool.tile([P, NPAD], F32)
            nc.vector.memset(c_sl[:], 0.0)
            h_sl = spool.tile([P, N], F32)
            nc.vector.memset(h_sl[:], 0.0)
            xi_f = spool.tile([P, 3, N], F32)
            xf_f = spool.tile([P, N], F32)
            c_ful = spool.tile([P, KC, max(TC + TN, 1)], BF16)
            xi_ful = spool.tile([P, KC, 3 * max(TN, 1)], BF16)
            xf_ful = spool.tile([P, KC, max(TN, 1)], BF16)

            ACT = mybir.ActivationFunctionType

            # ---- precompute xi/xf (row-sharded: this core's 128 rows) ------
            with nc.named_scope("xi_pre", notify=True):
                CCH = PSN
                for cc in range(0, N, CCH):
                    ncc = min(CCH, N - cc)
                    ps = pg.tile([P, 3, PSN], F32, tag="ps3")
                    for g in range(3):
                        for k in range(KX):
                            nc.tensor.matmul(
                                ps[:, g, :ncc],
                                iouxs[:, k * 3 + g, :],
                                xt[:, k, cc:cc + ncc],
                                start=(k == 0), stop=(k == KX - 1))
                    for g in range(3):
                        nc.scalar.activation(
                            xi_f[:, g, cc:cc + ncc], ps[:, g, :ncc],
                            ACT.Identity, bias=bxi[:, g:g + 1])
                    psf0 = pg.tile([P, K * NMAX], F32, tag="psf")
                    for k in range(KX):
                        nc.tensor.matmul(
                            psf0[:, :ncc], fxs[:, k, :], xt[:, k, cc:cc + ncc],
                            start=(k == 0), stop=(k == KX - 1))
                    nc.scalar.activation(
                        xf_f[:, cc:cc + ncc], psf0[:, :ncc],
                        ACT.Identity, bias=bxf[:, 0:1])

            def gates(p0, n, iou_ps, fh_src, ccg, nch):
                """Column-sharded gate math for parents at cols [p0, p0+n).
                iou_ps carries WSCALE*(iouh @ ch_sum); None for leaves."""
                NW = NMAX
                sfx = ""
                assert n <= NW
                if iou_ps is None:
                    gsrc = lambda g: xi_f[:, g, p0:p0 + n]
                else:
                    tmp = wk.tile([P, 3, NW], F32, tag="gtmp" + sfx)
                    nc.vector.tensor_scalar_mul(tmp[:, :, :n], iou_ps,
                                                1.0 / WSCALE)
                    nc.vector.tensor_add(tmp[:, :, :n], tmp[:, :, :n],
                                         xi_f[:, :, p0:p0 + n])
                    gsrc = lambda g: tmp[:, g, :n]
                ig = wk.tile([P, NW], F32, tag="ig" + sfx)
                og = wk.tile([P, NW], F32, tag="og" + sfx)
                ug = wk.tile([P, NW], F32, tag="ug" + sfx)
                nc.scalar.activation(ig[:, :n], gsrc(0), ACT.Sigmoid,
                                     bias=biou[:, 0:1])
                nc.scalar.activation(og[:, :n], gsrc(1), ACT.Sigmoid,
                                     bias=biou[:, 1:2])
                nc.scalar.activation(ug[:, :n], gsrc(2), ACT.Tanh,
                                     bias=biou[:, 2:3])
                cn = wk.tile([P, NW], F32, tag="cn" + sfx)
                nc.vector.tensor_mul(cn[:, :n], ig[:, :n], ug[:, :n])
                if fh_src is not None:
                    # f = sigmoid(fh + xf[parent] + b); fc = sum_k f*cc
                    fsb = wk.tile([P, K * NMAX], F32, tag="fsb")
                    xfb = xf_f[:, p0:p0 + n].rearrange(
                        "p (n one) -> p n one", one=1).broadcast_to((P, n, K))
                    nc.vector.tensor_add(
                        fsb[:, :nch].rearrange("p (n k) -> p n k", k=K),
                        fh_src.rearrange("p (n k) -> p n k", k=K), xfb)
                    nc.scalar.activation(fsb[:, :nch], fsb[:, :nch],
                                         ACT.Sigmoid, bias=bfh[:, 0:1])
                    nc.vector.tensor_mul(fsb[:, :nch], fsb[:, :nch],
                                         ccg[:, :nch])
                    fc = wk.tile([P, NMAX], F32, tag="fc")
                    nc.vector.tensor_reduce(
                        fc[:, :n],
                        fsb[:, :nch].rearrange("p (n k) -> p n k", k=K),
                        axis=mybir.AxisListType.X, op=mybir.AluOpType.add)
                    nc.vector.tensor_add(cn[:, :n], cn[:, :n], fc[:, :n])
                nc.vector.tensor_copy(c_sl[:, p0:p0 + n], cn[:, :n])
                tc_t = wk.tile([P, NW], F32, tag="tct" + sfx)
                nc.scalar.activation(tc_t[:, :n], cn[:, :n], ACT.Tanh)
                nc.vector.tensor_mul(h_sl[:, p0:p0 + n], og[:, :n], tc_t[:, :n])

            def publish_h(w, p0, n):
                """AllGather this wave's h cols; piggyback full-width c (and,
                on wave 0, bias-folded xi/xf of tail nodes) for the tail."""
                pig = pig_cols[w]
                npig = len(pig)
                nxi = TN * 4 if w == 0 and TN else 0
                tot = n + npig + nxi
                sfx = f"w{w}"
                hb = wk.tile([P, tot], BF16, tag="hb" + sfx)
                nc.vector.tensor_copy(hb[:, :n], h_sl[:, p0:p0 + n])
                for i, (cw, _) in enumerate(pig):
                    nc.vector.tensor_copy(hb[:, n + i:n + i + 1],
                                          c_sl[:, p0 + cw:p0 + cw + 1])
                if nxi:
                    # [3*TN xi(+biou) | TN xf(+bfh)], t-major xi triples
                    for i, nd in enumerate(tail_nodes):
                        cl = int(col_of[nd])
                        for g in range(3):
                            nc.vector.tensor_scalar_add(
                                hb[:, n + npig + 3 * i + g:n + npig + 3 * i + g + 1],
                                xi_f[:, g, cl:cl + 1], biou[:, g:g + 1])
                        nc.vector.tensor_scalar_add(
                            hb[:, n + npig + 3 * TN + i:n + npig + 3 * TN + i + 1],
                            xf_f[:, cl:cl + 1], bfh[:, 0:1])
                gin = dp.tile([P, tot], BF16, tag="gin" + sfx)
                nc.sync.dma_start(gin[:], hb[:])
                gout = dp.tile([NCORES, P, tot], BF16, tag="gout" + sfx,
                               addr_space="Shared")
                nc.gpsimd.collective_compute(
                    "AllGather", mybir.AluOpType.bypass,
                    ins=[gin.opt()], outs=[gout.opt()],
                    replica_groups=[list(range(NCORES))])
                nc.sync.dma_start(
                    h_bf[:, :, p0:p0 + n],
                    gout[:, :, :n].rearrange("k p n -> p k n"))
                if npig:
                    i = 0
                    while i < npig:
                        j = i + 1
                        while (j < npig and pig[j][1] == pig[j - 1][1] + 1
                               and pig[j][0] == pig[j - 1][0] + 1):
                            j += 1
                        _, fp0 = pig[i]
                        nc.sync.dma_start(
                            c_ful[:, :, fp0:fp0 + (j - i)],
                            gout[:, :, n + i:n + j].rearrange(
                                "k p n -> p k n"))
                        i = j
                if nxi:
                    nc.sync.dma_start(
                        xi_ful[:, :, :3 * TN],
                        gout[:, :, n + npig:n + npig + 3 * TN].rearrange(
                            "k p x -> p k x"))
                    nc.sync.dma_start(
                        xf_ful[:, :, :TN],
                        gout[:, :, n + npig + 3 * TN:n + npig + 4 * TN].rearrange(
                            "k p t -> p k t"))

            # ---- wave 0: leaves -------------------------------------------
            with nc.named_scope("leaves", notify=True):
                p0, p1 = waves[0]
                for cc in range(p0, p1, NMAX):
                    gates(cc, min(NMAX, p1 - cc), None, None, None, 0)
                publish_h(0, p0, p1 - p0)

            # ---- cooperative internal waves -------------------------------
            for wi in range(1, tail_w0):
                with nc.named_scope(f"wave{wi}", notify=True):
                    ns, per_core = wave_slots[wi - 1]
                    soff = wave_soff[wi - 1]
                    p0, p1 = waves[wi]
                    n = p1 - p0
                    nch = n * K
                    hch = wk.tile([P, KC, K * NMAX], BF16, tag="hch")
                    ccg = wk.tile([P, K * NMAX], F32, tag="ccg")
                    for (dst, src, ln) in wave_runs[wi - 1]:
                        nc.vector.tensor_copy(hch[:, :, dst:dst + ln],
                                              h_bf[:, :, src:src + ln])
                        nc.vector.tensor_copy(ccg[:, dst:dst + ln],
                                              c_sl[:, src:src + ln])
                    hsum_f = wk.tile([P, KC, NMAX], F32, tag="hsumf")
                    nc.vector.tensor_reduce(
                        hsum_f[:, :, :n],
                        hch[:, :, :nch].rearrange("p k (n c) -> p k n c", c=K),
                        axis=mybir.AxisListType.X, op=mybir.AluOpType.add)
                    hsum_b = wk.tile([P, KC, NMAX], BF16, tag="hsumb")
                    nc.vector.tensor_copy(hsum_b[:, :, :n], hsum_f[:, :, :n])

                    all_id = (ns == 0)
                    if not all_id:
                        ps1 = pp.tile([P, MC, PSN], F32, tag="ps1")
                        for s in range(ns):
                            hm = hmp.tile([P, KC, NMAX], BF16, tag="hm")
                            nc.vector.tensor_mul(
                                hm[:, :, :n], hsum_b[:, :, :n],
                                msk_sb[:, soff + s, :n].rearrange(
                                    "p (one n) -> p one n", one=1
                                ).broadcast_to((P, KC, n)))
                            wof = (soff + s) * MC * KC
                            for m in range(MC):
                                for k in range(KC):
                                    nc.tensor.matmul(
                                        ps1[:, m, :n],
                                        wres_sb[:, wof + m * KC + k, :],
                                        hm[:, k, :n],
                                        start=(s == 0 and k == 0 and m % 4 == 0),
                                        stop=(s == ns - 1 and k == KC - 1
                                              and m % 4 == 3))
                        cb = wk.tile([P, KC, n], BF16, tag="cb" + str(n))
                        nc.vector.tensor_copy(cb[:, :, :n], ps1[:, :, :n])
                        g1in = dp.tile([P, KC, n], BF16, tag="g1in" + str(n))
                        nc.sync.dma_start(g1in[:], cb[:])
                        g1out = dp.tile([P, KC, n], BF16, tag="g1out" + str(n),
                                        addr_space="Shared")
                        # disjoint masked contributions -> exact bf16 CCE adds
                        nc.gpsimd.collective_compute(
                            "AllReduce", mybir.AluOpType.add,
                            ins=[g1in.opt()], outs=[g1out.opt()],
                            replica_groups=[list(range(NCORES))])
                        chs_b = wk.tile([P, KC, n], BF16, tag="chsb" + str(n))
                        nc.sync.dma_start(chs_b[:], g1out[:])
                        rhs = chs_b
                    else:
                        hs16 = wk.tile([P, KC, NMAX], BF16, tag="hs16")
                        nc.vector.tensor_scalar_mul(
                            hs16[:, :, :n], hsum_b[:, :, :n], WSCALE)
                        rhs = hs16

                    # fh matmuls first: independent of the AllReduce, so the
                    # PE works on them while the collective runs
                    psf = pg.tile([P, K * NMAX], F32, tag="psf")
                    for k in range(KC):
                        nc.tensor.matmul(
                            psf[:, :nch], fhs[:, k, :], hch[:, k, :nch],
                            start=(k == 0), stop=(k == KC - 1))
                    psi = pg.tile([P, 3, PSN], F32, tag="ps3")
                    for g in range(3):
                        for k in range(KC):
                            nc.tensor.matmul(
                                psi[:, g, :n], iouhf[:, g * KC + k, :],
                                rhs[:, k, :n],
                                start=(k == 0), stop=(k == KC - 1))
                    gates(p0, n, psi[:, :, :n], psf[:, :nch], ccg, nch)
                    publish_h(wi, p0, n)

            # ---- replicated tail waves (no collectives) -------------------
            h_t = wk.tile([P, KC, max(TN, 1)], F32, tag="ht")
            t0c = waves[tail_w0][0] if tail_w0 < nwaves else 0
            TP = 16  # psum col pad for tail iou (keeps psit in one bank)
            for twi in range(nwaves - tail_w0):
                w = tail_w0 + twi
                with nc.named_scope(f"tail{w}", notify=True):
                    rl = tail_slots[twi]
                    p0, p1 = waves[w]
                    n = p1 - p0
                    nch = n * K
                    off = p0 - t0c
                    assert n <= TP and nch <= K * TAILN
                    hch = wk.tile([P, KC, K * NMAX], BF16, tag="hch")
                    for (dst, src, ln) in wave_runs[w - 1]:
                        nc.vector.tensor_copy(hch[:, :, dst:dst + ln],
                                              h_bf[:, :, src:src + ln])
                    ccg = wk.tile([P, KC, K * TAILN], F32, tag="ccgt")
                    nc.vector.memset(ccg[:, :, :nch], 0.0)
                    for j in range(p0, p1):
                        for kk, ch in enumerate(eff_children[order[j]]):
                            fp = cful_idx[int(col_of[ch])]
                            d0 = (j - p0) * K + kk
                            nc.vector.tensor_copy(ccg[:, :, d0:d0 + 1],
                                                  c_ful[:, :, fp:fp + 1])
                    hsum_f = wk.tile([P, KC, NMAX], F32, tag="hsumf")
                    nc.vector.tensor_reduce(
                        hsum_f[:, :, :n],
                        hch[:, :, :nch].rearrange("p k (n c) -> p k n c", c=K),
                        axis=mybir.AxisListType.X, op=mybir.AluOpType.add)
                    hsum_b = wk.tile([P, KC, NMAX], BF16, tag="hsumb")
                    nc.vector.tensor_copy(hsum_b[:, :, :n], hsum_f[:, :, :n])

                    if rl:
                        tso = tail_soff[twi]
                        ps1 = pp.tile([P, MC, PSN], F32, tag="ps1")
                        for s in range(len(rl)):
                            hm = hmp.tile([P, KC, NMAX], BF16, tag="hm")
                            nc.vector.tensor_mul(
                                hm[:, :, :n], hsum_b[:, :, :n],
                                msk_sb[:, tso + s, :n].rearrange(
                                    "p (one n) -> p one n", one=1
                                ).broadcast_to((P, KC, n)))
                            wof = (tso + s) * MC * KC
                            for m in range(MC):
                                for k in range(KC):
                                    nc.tensor.matmul(
                                        ps1[:, m, :n],
                                        wres_sb[:, wof + m * KC + k, :],
                                        hm[:, k, :n],
                                        start=(s == 0 and k == 0 and m % 4 == 0),
                                        stop=(s == len(rl) - 1 and k == KC - 1
                                              and m % 4 == 3))
                        chs = wk.tile([P, KC, TP], BF16, tag="chst")
                        nc.vector.tensor_copy(chs[:, :, :n], ps1[:, :, :n])
                        rhs = chs
                    else:
                        hs16 = wk.tile([P, KC, NMAX], BF16, tag="hs16")
                        nc.vector.tensor_scalar_mul(
                            hs16[:, :, :n], hsum_b[:, :, :n], WSCALE)
                        rhs = hs16

                    # full-width iou: one PSUM bank, single accumulation group
                    psit = pp.tile([P, KC, 3, TP], F32, tag="psit")
                    for m in range(KC):
                        for g in range(3):
                            for k in range(KC):
                                nc.tensor.matmul(
                                    psit[:, m, g, :n],
                                    iouhf[:, (m * 3 + g) * KC + k, :],
                                    rhs[:, k, :n],
                                    start=(m == 0 and g == 0 and k == 0),
                                    stop=(m == KC - 1 and g == 2 and k == KC - 1))
                    # full-width fh over child cols
                    psft = pp.tile([P, KC, K * TAILN], F32, tag="psft")
                    for m in range(KC):
                        for k in range(KC):
                            nc.tensor.matmul(
                                psft[:, m, :nch],
                                fhf[:, m * KC + k, :],
                                hch[:, k, :nch],
                                start=(m == 0 and k == 0),
                                stop=(m == KC - 1 and k == KC - 1))

                    IW = 1.0 / WSCALE
                    tmp = wk.tile([P, KC, 3, max(TN, 1)], F32, tag="gtmpt")
                    nc.vector.tensor_scalar_mul(tmp[:, :, :, :n],
                                                psit[:, :, :, :n], IW)
                    nc.vector.tensor_add(
                        tmp[:, :, :, :n], tmp[:, :, :, :n],
                        xi_ful[:, :, 3 * off:3 * (off + n)].rearrange(
                            "p c (t three) -> p c three t", three=3))
                    igt = wk.tile([P, KC, max(TN, 1)], F32, tag="igt")
                    ogt = wk.tile([P, KC, max(TN, 1)], F32, tag="ogt")
                    ugt = wk.tile([P, KC, max(TN, 1)], F32, tag="ugt")
                    nc.scalar.activation(igt[:, :, :n], tmp[:, :, 0, :n],
                                         ACT.Sigmoid)
                    nc.scalar.activation(ogt[:, :, :n], tmp[:, :, 1, :n],
                                         ACT.Sigmoid)
                    nc.scalar.activation(ugt[:, :, :n], tmp[:, :, 2, :n],
                                         ACT.Tanh)
                    cnt_ = wk.tile([P, KC, max(TN, 1)], F32, tag="cnt")
                    nc.vector.tensor_mul(cnt_[:, :, :n], igt[:, :, :n],
                                         ugt[:, :, :n])
                    fsb = wk.tile([P, KC, K * TAILN], F32, tag="fsbt")
                    nc.vector.tensor_scalar_mul(fsb[:, :, :nch],
                                                psft[:, :, :nch], 1.0 / WSCALE)
                    xfb = xf_ful[:, :, off:off + n].rearrange(
                        "p c (n one) -> p c n one", one=1).broadcast_to(
                        (P, KC, n, K))
                    nc.vector.tensor_add(
                        fsb[:, :, :nch].rearrange("p c (n k) -> p c n k", k=K),
                        fsb[:, :, :nch].rearrange("p c (n k) -> p c n k", k=K),
                        xfb)
                    nc.scalar.activation(fsb[:, :, :nch], fsb[:, :, :nch],
                                         ACT.Sigmoid)
                    nc.vector.tensor_mul(fsb[:, :, :nch], fsb[:, :, :nch],
                                         ccg[:, :, :nch])
                    fct = wk.tile([P, KC, max(TN, 1)], F32, tag="fct")
                    nc.vector.tensor_reduce(
                        fct[:, :, :n],
                        fsb[:, :, :nch].rearrange("p c (n k) -> p c n k", k=K),
                        axis=mybir.AxisListType.X, op=mybir.AluOpType.add)
                    nc.vector.tensor_add(cnt_[:, :, :n], cnt_[:, :, :n],
                                         fct[:, :, :n])
                    nc.vector.tensor_copy(c_ful[:, :, TC + off:TC + off + n],
                                          cnt_[:, :, :n])
                    tct = wk.tile([P, KC, max(TN, 1)], F32, tag="tctt")
                    nc.scalar.activation(tct[:, :, :n], cnt_[:, :, :n],
                                         ACT.Tanh)
                    nc.vector.tensor_mul(h_t[:, :, off:off + n],
                                         ogt[:, :, :n], tct[:, :, :n])
                    nc.vector.tensor_copy(h_bf[:, :, p0:p0 + n],
                                          h_t[:, :, off:off + n])

            nc.sync.dma_start(d_hout.ap(), h_sl[:])
            if TN:
                nc.sync.dma_start(d_tailh.ap(), h_t[:, :, :TN])

    in_maps = []
    for c in range(NCORES):
        in_maps.append({
            "wres": wres[c], "masks": masks_b[c],
            "xt": xT_b, "iouxstat": iouxstat[c],
            "fxstat": fxstat[c], "fhstat": fhstat[c],
            "iouh_full": iouh_full[c], "fh_full": fh_full,
            "b_xi": b_xi[c], "b_iou": b_iou[c], "b_xf": b_xf[c],
            "b_fh": b_fh[c],
        })
    _split_multi_waits(nc)
    meta = dict(col_of=col_of, tail_nodes=tail_nodes, N=N, MEM=MEM, TN=TN)
    return nc, in_maps, meta


def _assemble(meta, results):
    col_of, tail_nodes = meta["col_of"], meta["tail_nodes"]
    N, MEM, TN = meta["N"], meta["MEM"], meta["TN"]
    hT = np.concatenate([results[c]["hout"] for c in range(NCORES)], 0)
    out = np.empty((N, MEM), np.float32)
    for node in range(N):
        out[node] = hT[:, col_of[node]]
    if TN:
        th = results[0]["tailh"]  # [P, KC, TN]
        for i, nd in enumerate(tail_nodes):
            out[nd] = th[:, :, i].T.reshape(MEM)
    return out


def kernel(**inputs):
    nc, in_maps, meta = _build(inputs)
    kernel._nc = nc
    kernel._in_maps = in_maps
    res = run_bass_kernel_spmd(nc, in_maps, list(range(NCORES)))
    return _assemble(meta, [res.results[c] for c in range(NCORES)])


# revision 11
# speedup vs baseline: 1.1017x; 1.1017x over previous
"""ChildSumTreeLSTM with relation transforms on 8 Trainium2 NeuronCores.

Layout: everything transposed (features on SBUF partitions, tree nodes on the
free dim), node columns in topological-wave order (heap order). The h/c state
is row-sharded (each core owns a 128-row feature slice for every node), and
the relation matrices are k-chunk-sharded: core c holds the 128-column
contraction chunk c of EVERY relation matrix (fp8, scaled by 16), so the
per-wave relation matmul needs only the core's own h rows:
  local hsum (strided reduce over child cols of the core's own h slice) ->
  per-rel masked hsum copies -> 8 matmuls per rel accumulate a full-height
  partial ch_sum in one PSUM group -> bf16 AllReduce sums the k-chunk
  partials -> row-sharded iou/f gates -> AllGather publishes the new h
  columns (bf16, full height) for the f-gate matmuls and child gathers of
  later waves. The publish AllGather is OFF the critical path: the next
  wave's relation matmul needs only local data, so it overlaps the AG.
The last tiny waves run fully replicated on every core with full-width gate
weights (iouh bf16, merged with the row-sharded copy by per-core block
permutation; fh fp8), needing no collectives; full-width c / xi / xf for the
tail ride piggyback on the publish AllGathers; the host reads the tail
columns from core 0's separate full-width output.
All preload arrays are host-pre-transposed into SBUF layout (contiguous
per-partition DMA lines) and issued on the SP queue; latency-critical
runtime transfers (collective bounces etc.) use the Activation queue so they
are never stuck behind bulk preload triggers.
All per-core differences are input data, so one Bass program runs SPMD on
all 8 cores.
"""

import sys

sys.path.insert(0, "/opt/trn_rl_repo")

import numpy as np
import ml_dtypes

import concourse.bass as bass
import concourse.mybir as mybir
import concourse.tile as tile
from concourse.bass_utils import run_bass_kernel_spmd
from concourse.vector_clock import ScopedClock, VectorClock

BF16 = mybir.dt.bfloat16
F32 = mybir.dt.float32
FP8 = mybir.dt.float8e4
NCORES = 8
P = 128
WSCALE = 16.0
TAILN = 3   # waves with <= this many parents run replicated (no collectives)

# This walrus build rejects >1 sem wait per instruction at the Tile exit
# drain; split the aggregated drain into one drain per proc.
def _split_drain_and_barrier(self, tick_clock, wait_clock):
    gc = tick_clock.global_clock
    n = len(gc)
    nonzero = [i for i in range(n) if gc[i] > 0]
    for j in nonzero:
        vec = VectorClock([gc[i] if i == j else 0 for i in range(n)])
        d = self.nc.sync.drain()
        wait_clock.add_sem_waits(d.ins, ScopedClock({None: vec}))
    if not nonzero:
        d = self.nc.sync.drain()
        wait_clock.add_sem_waits(d.ins, ScopedClock({None: gc.copy()}))
    self.nc.all_engine_barrier()
    assert self.sems is not None
    popped = self.nc._tile_sem_poison_stack.pop()
    assert popped is self._sem_poison
    self.nc.clear_and_free_semaphores(list(self.sems.allocated().values()))
    self.nc.all_engine_barrier()


tile.TileContext._drain_and_barrier = _split_drain_and_barrier


def _split_multi_waits(nc, limit=1):
    """Walrus here allows only one sem wait per instruction; hoist extras
    onto same-engine NOPs inserted right before the instruction."""
    for bb in nc.main_func.blocks:
        new_list = []
        for ins in bb.instructions:
            si = getattr(ins, "sync_info", None)
            if si is not None and si.on_wait and len(si.on_wait) > limit:
                waits = list(si.on_wait)
                for w in waits[:-limit]:
                    nop = mybir.InstNoOp(
                        name=nc.get_next_instruction_name(),
                        sync_info=mybir.SyncInfo(on_wait=[w], on_update=[]),
                        bass_nofuse=True,
                        engine=ins.engine,
                    )
                    nc.register_instruction(nop, overwrite=True)
                    new_list.append(nop)
                si.on_wait = waits[-limit:]
            new_list.append(ins)
        bb.instructions[:] = new_list


def _bf16(a):
    return np.ascontiguousarray(a.astype(ml_dtypes.bfloat16))


def _fp8(a):
    return np.ascontiguousarray(a.astype(ml_dtypes.float8_e4m3))


def _blocksT(mat):
    """[M, K] -> [M/128 * K/128, 128, 128] of transposed blocks, k-major order
    grouped as [m, k] -> index m*KC + k, each block = mat[mb, kb].T (lhsT)."""
    M, K = mat.shape
    MC, KC = M // P, K // P
    out = np.empty((MC * KC, P, P), mat.dtype)
    for m in range(MC):
        for k in range(KC):
            out[m * KC + k] = mat[m * P:(m + 1) * P, k * P:(k + 1) * P].T
    return out


def _plan(child_idx, rel_ids, Wrel):
    """Host-side planning: waves, column order, rel->core assignment, slots."""
    N, K = child_idx.shape
    eff_children = []
    wave = np.zeros(N, np.int32)
    for i in range(N):
        cs = [int(c) for c in child_idx[i] if 0 <= c < i]
        eff_children.append(cs)
        wave[i] = 1 + max((wave[c] for c in cs), default=-1)
    nwaves = int(wave.max()) + 1
    # column order: by (wave, descending node) -> for the reference heap tree
    # this is exactly heap order, keeping children of consecutive parents
    # contiguous.
    order = sorted(range(N), key=lambda i: (wave[i], -i))
    col_of = np.empty(N, np.int64)
    for j, node in enumerate(order):
        col_of[node] = j
    waves = []  # list of (p0, p1) col ranges
    j = 0
    for w in range(nwaves):
        cnt = int((wave == w).sum())
        waves.append((j, j + cnt))
        j += cnt

    ident = set()
    eye = np.eye(Wrel.shape[1], dtype=Wrel.dtype)
    for r in set(int(rel_ids[i]) for i in range(N)):
        if np.array_equal(Wrel[r], eye):
            ident.add(r)

    # first tail wave: every wave from tail_w0 on is tiny and runs replicated
    tail_w0 = nwaves
    while tail_w0 - 1 >= 1 and waves[tail_w0 - 1][1] - waves[tail_w0 - 1][0] <= TAILN:
        tail_w0 -= 1

    # per cooperative wave (1..tail_w0-1): rels present; identity-only waves
    # skip the relation matmul entirely
    wave_rels = []
    for w in range(1, tail_w0):
        p0, p1 = waves[w]
        rels_all = set(int(rel_ids[order[j]]) for j in range(p0, p1))
        if rels_all <= ident:
            wave_rels.append([])
        else:
            wave_rels.append(sorted(rels_all))

    # distinct coop rels ordered by first-use wave (for DMA priority)
    rel_order = []
    for rels in wave_rels:
        for r in rels:
            if r not in rel_order:
                rel_order.append(r)

    # tail waves: replicated slots, one per non-identity rel per wave
    tail_slots = []
    for w in range(tail_w0, nwaves):
        p0, p1 = waves[w]
        rels_all = set(int(rel_ids[order[j]]) for j in range(p0, p1))
        if rels_all <= ident:
            tail_slots.append([])
        else:
            tail_slots.append(sorted(rels_all))

    # cols in cooperative waves whose full-width c a tail wave needs
    tail_child_cols = sorted(set(
        int(col_of[c]) for w in range(tail_w0, nwaves)
        for j in range(waves[w][0], waves[w][1])
        for c in eff_children[order[j]]
        if wave[c] < tail_w0))

    return dict(order=order, col_of=col_of, waves=waves, wave=wave,
                eff_children=eff_children, ident=ident,
                wave_rels=wave_rels, rel_order=rel_order,
                nwaves=nwaves, tail_w0=tail_w0,
                tail_slots=tail_slots, tail_child_cols=tail_child_cols)


def _build(inputs):
    x = np.asarray(inputs["x"], np.float32)
    Wrel = np.asarray(inputs["Wrel"], np.float32)
    ioux_w = np.asarray(inputs["ioux_w"], np.float32)
    ioux_b = np.asarray(inputs["ioux_b"], np.float32)
    iouh_w = np.asarray(inputs["iouh_w"], np.float32)
    iouh_b = np.asarray(inputs["iouh_b"], np.float32)
    fx_w = np.asarray(inputs["fx_w"], np.float32)
    fx_b = np.asarray(inputs["fx_b"], np.float32)
    fh_w = np.asarray(inputs["fh_w"], np.float32)
    fh_b = np.asarray(inputs["fh_b"], np.float32)
    child_idx = np.asarray(inputs["child_idx"], np.int32)
    rel_ids = np.asarray(inputs["rel_ids"], np.int32)

    N, IN_DIM = x.shape
    MEM = fh_w.shape[0]
    KC = MEM // P           # 8 feature chunks
    KX = IN_DIM // P        # 8 input chunks
    K = child_idx.shape[1]  # max children (4)
    NPAD = N + K + 4
    assert KC == NCORES  # publish-AG rank axis doubles as the row-chunk axis

    plan = _plan(child_idx, rel_ids, Wrel)
    order, col_of, waves = plan["order"], plan["col_of"], plan["waves"]
    eff_children = plan["eff_children"]
    wave_rels, rel_order = plan["wave_rels"], plan["rel_order"]
    nwaves = plan["nwaves"]
    tail_w0, tail_slots = plan["tail_w0"], plan["tail_slots"]
    tail_child_cols = plan["tail_child_cols"]

    # Child gather plan: per internal wave, the flattened (parent-major)
    # child column sequence, decomposed into maximal +1-contiguous runs.
    ZCOL = N
    child_col = np.full((N, K), ZCOL, np.int64)
    for i in range(N):
        for kk, c in enumerate(eff_children[i]):
            child_col[i, kk] = col_of[c]
    wave_runs = []
    for w in range(1, nwaves):
        p0, p1 = waves[w]
        seq = []
        for j in range(p0, p1):
            seq.extend(child_col[order[j]])
        runs = []
        i0 = 0
        while i0 < len(seq):
            i1 = i0 + 1
            while i1 < len(seq) and seq[i1] == seq[i1 - 1] + 1:
                i1 += 1
            runs.append((i0, int(seq[i0]), i1 - i0))
            i0 = i1
        wave_runs.append(runs)

    # tail bookkeeping
    TC = len(tail_child_cols)
    tail_nodes = [order[j] for w in range(tail_w0, nwaves)
                  for j in range(waves[w][0], waves[w][1])]
    TN = len(tail_nodes)
    cful_idx = {c: i for i, c in enumerate(tail_child_cols)}
    for i, nd in enumerate(tail_nodes):
        cful_idx[int(col_of[nd])] = TC + i
    pig_cols = []  # per wave 0..tail_w0-1: list of (col_in_wave, cful_pos)
    for w in range(0, tail_w0):
        p0, p1 = waves[w]
        pig_cols.append([(j - p0, cful_idx[j]) for j in range(p0, p1)
                         if j in cful_idx])

    # ---- per-core host data (all pre-transposed into SBUF layout) ----------
    xT = np.ascontiguousarray(x[order].T)
    xt_h = np.zeros((P, KX, N), ml_dtypes.bfloat16)
    for k in range(KX):
        xt_h[:, k, :] = _bf16(xT[k * P:(k + 1) * P])

    NREL = len(rel_order)
    rel_idx = {r: i for i, r in enumerate(rel_order)}
    NMAX = max((waves[w][1] - waves[w][0]) for w in range(1, nwaves)) if nwaves > 1 else 1
    NBIG = max(p1 - p0 for p0, p1 in waves)
    PSN = 128  # psum column pad so each m-chunk slice stays inside one bank
    assert NMAX <= PSN and K * NMAX <= 512
    MC = MEM // P

    # k-chunk relation weights: core c holds lhsT blocks of W_r[:, c*128:...]
    # wsk_h[c][p, ri*MC + m, q] = (W_r * WSCALE)[m*128+q, c*128+p]
    wsk_h = [np.zeros((P, max(NREL, 1) * MC, P), ml_dtypes.float8_e4m3)
             for _ in range(NCORES)]
    for ri, r in enumerate(rel_order):
        Ws = (Wrel[r] * WSCALE).astype(ml_dtypes.float8_e4m3)
        for c in range(NCORES):
            blk = Ws[:, c * P:(c + 1) * P]           # [MEM, P]
            for m in range(MC):
                wsk_h[c][:, ri * MC + m, :] = blk[m * P:(m + 1) * P, :].T
    # per-(wave, rel) column masks, replicated over partitions
    mask_rows = []  # (wave, rel) -> row index
    mrow = {}
    for wi in range(1, tail_w0):
        p0, p1 = waves[wi]
        for r in wave_rels[wi - 1]:
            mrow[(wi, r)] = len(mask_rows)
            row = np.zeros(NMAX, np.float32)
            for t in range(p1 - p0):
                if int(rel_ids[order[p0 + t]]) == r:
                    row[t] = 1.0
            mask_rows.append(row)
    # tail replicated slots: full 64-block matrices + masks
    S_tail = sum(len(rl) for rl in tail_slots)
    wst_h = np.zeros((P, max(S_tail, 1) * MC * KC, P), ml_dtypes.float8_e4m3)
    tail_soff = []
    ts_i = 0
    for twi, rl in enumerate(tail_slots):
        w = tail_w0 + twi
        p0, p1 = waves[w]
        tail_soff.append(ts_i)
        for r in rl:
            blkT = (_blocksT(Wrel[r]) * WSCALE).astype(ml_dtypes.float8_e4m3)
            wst_h[:, ts_i * MC * KC:(ts_i + 1) * MC * KC, :] = \
                blkT.transpose(1, 0, 2)
            row = np.zeros(NMAX, np.float32)
            for t in range(p1 - p0):
                if int(rel_ids[order[p0 + t]]) == r:
                    row[t] = 1.0
            mrow[(w, r)] = len(mask_rows)
            mask_rows.append(row)
            ts_i += 1
    NM = max(len(mask_rows), 1)
    mask_h = np.ascontiguousarray(np.broadcast_to(
        np.stack(mask_rows) if mask_rows else np.zeros((1, NMAX), np.float32),
        (P, NM, NMAX)).astype(ml_dtypes.bfloat16))

    iouxs_h = [np.zeros((P, KX * 3, P), ml_dtypes.bfloat16) for _ in range(NCORES)]
    fxs_h = [np.zeros((P, KX, P), ml_dtypes.bfloat16) for _ in range(NCORES)]
    fhs_h = [np.zeros((P, KC, P), ml_dtypes.bfloat16) for _ in range(NCORES)]
    b_xi = [np.zeros((3, P), np.float32) for _ in range(NCORES)]
    b_iou = [np.zeros((3, P), np.float32) for _ in range(NCORES)]
    b_xf = [np.zeros((P,), np.float32) for _ in range(NCORES)]
    b_fh = [np.zeros((P,), np.float32) for _ in range(NCORES)]
    for c in range(NCORES):
        rows = slice(c * P, (c + 1) * P)
        for g in range(3):
            gr = slice(g * MEM + c * P, g * MEM + (c + 1) * P)
            b_xi[c][g] = ioux_b[gr]
            b_iou[c][g] = iouh_b[gr]
            for k in range(KX):
                iouxs_h[c][:, k * 3 + g, :] = _bf16(
                    ioux_w[gr, k * P:(k + 1) * P].T)
        b_xf[c] = fx_b[rows]
        b_fh[c] = fh_b[rows]
        for k in range(KX):
            fxs_h[c][:, k, :] = _bf16(fx_w[rows, k * P:(k + 1) * P].T)
        for k in range(KC):
            fhs_h[c][:, k, :] = _bf16(fh_w[rows, k * P:(k + 1) * P].T)
    bxi_h = [np.ascontiguousarray(b.T.astype(np.float32)) for b in b_xi]
    biou_h = [np.ascontiguousarray(b.T.astype(np.float32)) for b in b_iou]
    bxf_h = [b.reshape(P, 1).astype(np.float32) for b in b_xf]
    bfh_h = [b.reshape(P, 1).astype(np.float32) for b in b_fh]

    # full-width iouh in bf16, per-core block order: m-slot-major with the
    # core's own row-chunk in slot 0 (serves row-sharded coop iou via slot 0
    # and the replicated tail via all slots; tail rows permuted on cores
    # != 0, whose tail output is unused). SBUF layout [P, (ms*3+g)*KC+k, P].
    iouh_blk = _blocksT(iouh_w)  # [(g*KC+m)*KC+k, P, P]
    iouhf_h = []
    for c in range(NCORES):
        sig = [c] + [m for m in range(KC) if m != c]
        buf = np.zeros((P, KC * 3 * KC, P), ml_dtypes.bfloat16)
        for ms in range(KC):
            for g in range(3):
                for k in range(KC):
                    buf[:, (ms * 3 + g) * KC + k, :] = _bf16(
                        iouh_blk[(g * KC + sig[ms]) * KC + k])
        iouhf_h.append(buf)
    fhf_h = np.ascontiguousarray(
        _fp8(_blocksT(fh_w * WSCALE)).transpose(1, 0, 2))  # [P, KC*KC, P]

    # ---- build program ------------------------------------------------------
    nc = bass.Bass("TRN2", target_bir_lowering=False, debug=False,
                   num_devices=NCORES)
    d_wsk = nc.dram_tensor("wsk", list(wsk_h[0].shape), FP8,
                           kind="ExternalInput")
    d_wst = nc.dram_tensor("wst", list(wst_h.shape), FP8,
                           kind="ExternalInput")
    d_mask = nc.dram_tensor("masks", list(mask_h.shape), BF16,
                            kind="ExternalInput")
    d_xt = nc.dram_tensor("xt", [P, KX, N], BF16, kind="ExternalInput")
    d_iouxs = nc.dram_tensor("iouxstat", [P, KX * 3, P], BF16, kind="ExternalInput")
    d_fxs = nc.dram_tensor("fxstat", [P, KX, P], BF16, kind="ExternalInput")
    d_fhs = nc.dram_tensor("fhstat", [P, KC, P], BF16, kind="ExternalInput")
    d_iouhf = nc.dram_tensor("iouh_full", [P, KC * 3 * KC, P], BF16,
                             kind="ExternalInput")
    d_fhf = nc.dram_tensor("fh_full", [P, KC * KC, P], FP8,
                           kind="ExternalInput")
    d_bxi = nc.dram_tensor("b_xi", [P, 3], F32, kind="ExternalInput")
    d_biou = nc.dram_tensor("b_iou", [P, 3], F32, kind="ExternalInput")
    d_bxf = nc.dram_tensor("b_xf", [P, 1], F32, kind="ExternalInput")
    d_bfh = nc.dram_tensor("b_fh", [P, 1], F32, kind="ExternalInput")
    d_hout = nc.dram_tensor("hout", [P, N], F32, kind="ExternalOutput")
    d_tailh = nc.dram_tensor("tailh", [P, KC, max(TN, 1)], F32,
                             kind="ExternalOutput")

    # rels needed by wave 1 (DMA'd first), then the rest
    r_w1 = len(wave_rels[0]) if wave_rels else 0

    with tile.TileContext(nc, num_cores=NCORES) as tc:
        with (
            tc.tile_pool(name="const", bufs=1) as cpool,
            tc.tile_pool(name="state", bufs=1) as spool,
            tc.tile_pool(name="hmp", bufs=3) as hmp,
            tc.tile_pool(name="work", bufs=1) as wk,
            tc.tile_pool(name="psum", bufs=1, space="PSUM") as pp,
            tc.tile_pool(name="psg", bufs=2, space="PSUM") as pg,
            tc.tile_pool(name="dram", bufs=2, space="DRAM") as dp,
        ):
            # bulk preloads on the SP queue, in priority order; all host
            # arrays are already in SBUF layout (contiguous partition lines)
            xt = cpool.tile([P, KX, N], BF16)
            nc.sync.dma_start(xt[:], d_xt.ap())
            iouxs = cpool.tile([P, KX * 3, P], BF16)
            nc.sync.dma_start(iouxs[:], d_iouxs.ap())
            fxs = cpool.tile([P, KX, P], BF16)
            nc.sync.dma_start(fxs[:], d_fxs.ap())
            bxi = cpool.tile([P, 3], F32)
            nc.sync.dma_start(bxi[:], d_bxi.ap())
            biou = cpool.tile([P, 3], F32)
            nc.sync.dma_start(biou[:], d_biou.ap())
            bxf = cpool.tile([P, 1], F32)
            nc.sync.dma_start(bxf[:], d_bxf.ap())
            bfh = cpool.tile([P, 1], F32)
            nc.sync.dma_start(bfh[:], d_bfh.ap())

            wsk = cpool.tile([P, max(NREL, 1) * MC, P], FP8)
            msk_sb = cpool.tile([P, NM, NMAX], BF16)
            fhs = cpool.tile([P, KC, P], BF16)
            iouhf = cpool.tile([P, KC * 3 * KC, P], BF16)
            fhf = cpool.tile([P, KC * KC, P], FP8)
            wst = cpool.tile([P, max(S_tail, 1) * MC * KC, P], FP8)

            # wave-1 rels + masks first, then row-sharded gate weights, then
            # everything else in use order
            if NREL:
                nc.sync.dma_start(wsk[:, :r_w1 * MC, :],
                                  d_wsk.ap()[:, :r_w1 * MC])
            nc.sync.dma_start(msk_sb[:], d_mask.ap())
            nc.sync.dma_start(iouhf[:, :3 * KC, :], d_iouhf.ap()[:, :3 * KC])
            nc.sync.dma_start(fhs[:], d_fhs.ap())
            if NREL > r_w1:
                nc.sync.dma_start(wsk[:, r_w1 * MC:, :],
                                  d_wsk.ap()[:, r_w1 * MC:])
            if S_tail:
                nc.sync.dma_start(wst[:], d_wst.ap())
            nc.sync.dma_start(iouhf[:, 3 * KC:, :], d_iouhf.ap()[:, 3 * KC:])
            nc.sync.dma_start(fhf[:], d_fhf.ap())

            # state
            h_bf = spool.tile([P, KC, NPAD], BF16)
            nc.vector.memset(h_bf[:], 0.0)
            c_sl = spool.tile([P, NPAD], F32)
            nc.vector.memset(c_sl[:], 0.0)
            h_sl = spool.tile([P, NPAD], F32)
            nc.vector.memset(h_sl[:], 0.0)
            xi_f = spool.tile([P, 3, N], F32)
            xf_f = spool.tile([P, N], F32)
            c_ful = spool.tile([P, KC, max(TC + TN, 1)], BF16)
            xi_ful = spool.tile([P, KC, 3 * max(TN, 1)], BF16)
            xf_ful = spool.tile([P, KC, max(TN, 1)], BF16)

            ACT = mybir.ActivationFunctionType

            # ---- precompute xi/xf (row-sharded: this core's 128 rows) ------
            with nc.named_scope("xi_pre", notify=True):
                CCH = PSN
                for cc in range(0, N, CCH):
                    ncc = min(CCH, N - cc)
                    ps = pg.tile([P, 3, PSN], F32, tag="ps3")
                    for g in range(3):
                        for k in range(KX):
                            nc.tensor.matmul(
                                ps[:, g, :ncc],
                                iouxs[:, k * 3 + g, :],
                                xt[:, k, cc:cc + ncc],
                                start=(k == 0), stop=(k == KX - 1))
                    for g in range(3):
                        nc.scalar.activation(
                            xi_f[:, g, cc:cc + ncc], ps[:, g, :ncc],
                            ACT.Identity, bias=bxi[:, g:g + 1])
                    psf0 = pg.tile([P, K * NMAX], F32, tag="psf")
                    for k in range(KX):
                        nc.tensor.matmul(
                            psf0[:, :ncc], fxs[:, k, :], xt[:, k, cc:cc + ncc],
                            start=(k == 0), stop=(k == KX - 1))
                    nc.scalar.activation(
                        xf_f[:, cc:cc + ncc], psf0[:, :ncc],
                        ACT.Identity, bias=bxf[:, 0:1])

            def gates(p0, n, iou_ps, fh_src, ccg, nch):
                """Row-sharded gate math for parents at cols [p0, p0+n).
                iou_ps carries WSCALE*(iouh @ ch_sum); None for leaves."""
                NW = NMAX
                sfx = ""
                assert n <= NW
                if iou_ps is None:
                    gsrc = lambda g: xi_f[:, g, p0:p0 + n]
                else:
                    tmp = wk.tile([P, 3, NW], F32, tag="gtmp" + sfx)
                    nc.vector.tensor_scalar_mul(tmp[:, :, :n], iou_ps,
                                                1.0 / WSCALE)
                    nc.vector.tensor_add(tmp[:, :, :n], tmp[:, :, :n],
                                         xi_f[:, :, p0:p0 + n])
                    gsrc = lambda g: tmp[:, g, :n]
                ig = wk.tile([P, NW], F32, tag="ig" + sfx)
                og = wk.tile([P, NW], F32, tag="og" + sfx)
                ug = wk.tile([P, NW], F32, tag="ug" + sfx)
                nc.scalar.activation(ig[:, :n], gsrc(0), ACT.Sigmoid,
                                     bias=biou[:, 0:1])
                nc.scalar.activation(og[:, :n], gsrc(1), ACT.Sigmoid,
                                     bias=biou[:, 1:2])
                nc.scalar.activation(ug[:, :n], gsrc(2), ACT.Tanh,
                                     bias=biou[:, 2:3])
                cn = wk.tile([P, NW], F32, tag="cn" + sfx)
                nc.vector.tensor_mul(cn[:, :n], ig[:, :n], ug[:, :n])
                if fh_src is not None:
                    # f = sigmoid(fh + xf[parent] + b); fc = sum_k f*cc
                    fsb = wk.tile([P, K * NMAX], F32, tag="fsb")
                    xfb = xf_f[:, p0:p0 + n].rearrange(
                        "p (n one) -> p n one", one=1).broadcast_to((P, n, K))
                    nc.vector.tensor_add(
                        fsb[:, :nch].rearrange("p (n k) -> p n k", k=K),
                        fh_src.rearrange("p (n k) -> p n k", k=K), xfb)
                    nc.scalar.activation(fsb[:, :nch], fsb[:, :nch],
                                         ACT.Sigmoid, bias=bfh[:, 0:1])
                    nc.vector.tensor_mul(fsb[:, :nch], fsb[:, :nch],
                                         ccg[:, :nch])
                    fc = wk.tile([P, NMAX], F32, tag="fc")
                    nc.vector.tensor_reduce(
                        fc[:, :n],
                        fsb[:, :nch].rearrange("p (n k) -> p n k", k=K),
                        axis=mybir.AxisListType.X, op=mybir.AluOpType.add)
                    nc.vector.tensor_add(cn[:, :n], cn[:, :n], fc[:, :n])
                nc.vector.tensor_copy(c_sl[:, p0:p0 + n], cn[:, :n])
                tc_t = wk.tile([P, NW], F32, tag="tct" + sfx)
                nc.scalar.activation(tc_t[:, :n], cn[:, :n], ACT.Tanh)
                nc.vector.tensor_mul(h_sl[:, p0:p0 + n], og[:, :n], tc_t[:, :n])

            def publish_h(w, p0, n):
                """AllGather this wave's h cols; piggyback full-width c (and,
                on wave 0, bias-folded xi/xf of tail nodes) for the tail.
                Bounce DMAs ride the Activation queue."""
                pig = pig_cols[w]
                npig = len(pig)
                nxi = TN * 4 if w == 0 and TN else 0
                tot = n + npig + nxi
                sfx = f"w{w}"
                hb = wk.tile([P, tot], BF16, tag="hb" + sfx)
                nc.vector.tensor_copy(hb[:, :n], h_sl[:, p0:p0 + n])
                for i, (cw, _) in enumerate(pig):
                    nc.vector.tensor_copy(hb[:, n + i:n + i + 1],
                                          c_sl[:, p0 + cw:p0 + cw + 1])
                if nxi:
                    # [3*TN xi(+biou) | TN xf(+bfh)], t-major xi triples
                    for i, nd in enumerate(tail_nodes):
                        cl = int(col_of[nd])
                        for g in range(3):
                            nc.vector.tensor_scalar_add(
                                hb[:, n + npig + 3 * i + g:n + npig + 3 * i + g + 1],
                                xi_f[:, g, cl:cl + 1], biou[:, g:g + 1])
                        nc.vector.tensor_scalar_add(
                            hb[:, n + npig + 3 * TN + i:n + npig + 3 * TN + i + 1],
                            xf_f[:, cl:cl + 1], bfh[:, 0:1])
                gin = dp.tile([P, tot], BF16, tag="gin" + sfx)
                nc.scalar.dma_start(gin[:], hb[:])
                gout = dp.tile([NCORES, P, tot], BF16, tag="gout" + sfx,
                               addr_space="Shared")
                nc.gpsimd.collective_compute(
                    "AllGather", mybir.AluOpType.bypass,
                    ins=[gin.opt()], outs=[gout.opt()],
                    replica_groups=[list(range(NCORES))])
                nc.scalar.dma_start(
                    h_bf[:, :, p0:p0 + n],
                    gout[:, :, :n].rearrange("k p n -> p k n"))
                if npig:
                    i = 0
                    while i < npig:
                        j = i + 1
                        while (j < npig and pig[j][1] == pig[j - 1][1] + 1
                               and pig[j][0] == pig[j - 1][0] + 1):
                            j += 1
                        _, fp0 = pig[i]
                        nc.scalar.dma_start(
                            c_ful[:, :, fp0:fp0 + (j - i)],
                            gout[:, :, n + i:n + j].rearrange(
                                "k p n -> p k n"))
                        i = j
                if nxi:
                    nc.scalar.dma_start(
                        xi_ful[:, :, :3 * TN],
                        gout[:, :, n + npig:n + npig + 3 * TN].rearrange(
                            "k p x -> p k x"))
                    nc.scalar.dma_start(
                        xf_ful[:, :, :TN],
                        gout[:, :, n + npig + 3 * TN:n + npig + 4 * TN].rearrange(
                            "k p t -> p k t"))

            # ---- wave 0: leaves -------------------------------------------
            with nc.named_scope("leaves", notify=True):
                p0, p1 = waves[0]
                for cc in range(p0, p1, NMAX):
                    gates(cc, min(NMAX, p1 - cc), None, None, None, 0)
                publish_h(0, p0, p1 - p0)

            # ---- cooperative internal waves -------------------------------
            for wi in range(1, tail_w0):
                with nc.named_scope(f"wave{wi}", notify=True):
                    rels = wave_rels[wi - 1]
                    p0, p1 = waves[wi]
                    n = p1 - p0
                    nch = n * K
                    # local hsum from the core's own h rows (h_sl)
                    hcs = wk.tile([P, K * NMAX], F32, tag="hcs")
                    for (dst, src, ln) in wave_runs[wi - 1]:
                        nc.vector.tensor_copy(hcs[:, dst:dst + ln],
                                              h_sl[:, src:src + ln])
                    hsum = wk.tile([P, NMAX], F32, tag="hsum")
                    nc.vector.tensor_reduce(
                        hsum[:, :n],
                        hcs[:, :nch].rearrange("p (n c) -> p n c", c=K),
                        axis=mybir.AxisListType.X, op=mybir.AluOpType.add)

                    if rels:
                        # k-chunk partial ch_sum over all rels, one PSUM group
                        ps1 = pp.tile([P, MC, PSN], F32, tag="ps1")
                        for ri, r in enumerate(rels):
                            hm = hmp.tile([P, NMAX], BF16, tag="hm")
                            nc.vector.tensor_mul(
                                hm[:, :n], hsum[:, :n],
                                msk_sb[:, mrow[(wi, r)], :n])
                            gi = rel_idx[r]
                            for m in range(MC):
                                nc.tensor.matmul(
                                    ps1[:, m, :n],
                                    wsk[:, gi * MC + m, :],
                                    hm[:, :n],
                                    start=(ri == 0 and m % 4 == 0),
                                    stop=(ri == len(rels) - 1 and m % 4 == 3))
                        cb = wk.tile([P, KC, n], BF16, tag="cb" + str(n))
                        nc.vector.tensor_copy(cb[:, :, :n], ps1[:, :, :n])
                        g1in = dp.tile([P, KC, n], BF16, tag="g1in" + str(n))
                        nc.scalar.dma_start(g1in[:], cb[:])
                        g1out = dp.tile([P, KC, n], BF16, tag="g1out" + str(n),
                                        addr_space="Shared")
                        nc.gpsimd.collective_compute(
                            "AllReduce", mybir.AluOpType.add,
                            ins=[g1in.opt()], outs=[g1out.opt()],
                            replica_groups=[list(range(NCORES))])
                        chs_b = wk.tile([P, KC, n], BF16, tag="chsb" + str(n))
                        nc.scalar.dma_start(chs_b[:], g1out[:])
                        rhs = chs_b
                    else:
                        # identity wave: ch_sum == hsum needs full height;
                        # gather from published h and scale by WSCALE
                        hchf = wk.tile([P, KC, K * NMAX], BF16, tag="hch")
                        for (dst, src, ln) in wave_runs[wi - 1]:
                            nc.vector.tensor_copy(hchf[:, :, dst:dst + ln],
                                                  h_bf[:, :, src:src + ln])
                        hsf = wk.tile([P, KC, NMAX], F32, tag="hsumf")
                        nc.vector.tensor_reduce(
                            hsf[:, :, :n],
                            hchf[:, :, :nch].rearrange(
                                "p k (n c) -> p k n c", c=K),
                            axis=mybir.AxisListType.X, op=mybir.AluOpType.add)
                        hs16 = wk.tile([P, KC, NMAX], BF16, tag="hs16")
                        nc.vector.tensor_scalar_mul(
                            hs16[:, :, :n], hsf[:, :, :n], WSCALE)
                        rhs = hs16

                    # gather full-height child h (for fh) and own-row c
                    hch = wk.tile([P, KC, K * NMAX], BF16, tag="hch")
                    ccg = wk.tile([P, K * NMAX], F32, tag="ccg")
                    for (dst, src, ln) in wave_runs[wi - 1]:
                        nc.vector.tensor_copy(hch[:, :, dst:dst + ln],
                                              h_bf[:, :, src:src + ln])
                        nc.vector.tensor_copy(ccg[:, dst:dst + ln],
                                              c_sl[:, src:src + ln])
                    # fh matmuls first: independent of the AllReduce
                    psf = pg.tile([P, K * NMAX], F32, tag="psf")
                    for k in range(KC):
                        nc.tensor.matmul(
                            psf[:, :nch], fhs[:, k, :], hch[:, k, :nch],
                            start=(k == 0), stop=(k == KC - 1))
                    psi = pg.tile([P, 3, PSN], F32, tag="ps3")
                    for g in range(3):
                        for k in range(KC):
                            nc.tensor.matmul(
                                psi[:, g, :n], iouhf[:, g * KC + k, :],
                                rhs[:, k, :n],
                                start=(k == 0), stop=(k == KC - 1))
                    gates(p0, n, psi[:, :, :n], psf[:, :nch], ccg, nch)
                    publish_h(wi, p0, n)

            # ---- replicated tail waves (no collectives) -------------------
            h_t = wk.tile([P, KC, max(TN, 1)], F32, tag="ht")
            t0c = waves[tail_w0][0] if tail_w0 < nwaves else 0
            TP = 16  # psum col pad for tail iou (keeps psit in one bank)
            for twi in range(nwaves - tail_w0):
                w = tail_w0 + twi
                with nc.named_scope(f"tail{w}", notify=True):
                    rl = tail_slots[twi]
                    p0, p1 = waves[w]
                    n = p1 - p0
                    nch = n * K
                    off = p0 - t0c
                    assert n <= TP and nch <= K * TAILN
                    hch = wk.tile([P, KC, K * NMAX], BF16, tag="hch")
                    for (dst, src, ln) in wave_runs[w - 1]:
                        nc.vector.tensor_copy(hch[:, :, dst:dst + ln],
                                              h_bf[:, :, src:src + ln])
                    ccg = wk.tile([P, KC, K * TAILN], F32, tag="ccgt")
                    nc.vector.memset(ccg[:, :, :nch], 0.0)
                    for j in range(p0, p1):
                        for kk, ch in enumerate(eff_children[order[j]]):
                            fp = cful_idx[int(col_of[ch])]
                            d0 = (j - p0) * K + kk
                            nc.vector.tensor_copy(ccg[:, :, d0:d0 + 1],
                                                  c_ful[:, :, fp:fp + 1])
                    hsum_f = wk.tile([P, KC, NMAX], F32, tag="hsumf")
                    nc.vector.tensor_reduce(
                        hsum_f[:, :, :n],
                        hch[:, :, :nch].rearrange("p k (n c) -> p k n c", c=K),
                        axis=mybir.AxisListType.X, op=mybir.AluOpType.add)
                    hsum_b = wk.tile([P, KC, NMAX], BF16, tag="hsumb")
                    nc.vector.tensor_copy(hsum_b[:, :, :n], hsum_f[:, :, :n])

                    if rl:
                        tso = tail_soff[twi]
                        ps1 = pp.tile([P, MC, PSN], F32, tag="ps1")
                        for s, r in enumerate(rl):
                            hm = hmp.tile([P, KC, NMAX], BF16, tag="hmt")
                            nc.vector.tensor_mul(
                                hm[:, :, :n], hsum_b[:, :, :n],
                                msk_sb[:, mrow[(w, r)], :n].rearrange(
                                    "p (one n) -> p one n", one=1
                                ).broadcast_to((P, KC, n)))
                            wof = (tso + s) * MC * KC
                            for m in range(MC):
                                for k in range(KC):
                                    nc.tensor.matmul(
                                        ps1[:, m, :n],
                                        wst[:, wof + m * KC + k, :],
                                        hm[:, k, :n],
                                        start=(s == 0 and k == 0 and m % 4 == 0),
                                        stop=(s == len(rl) - 1 and k == KC - 1
                                              and m % 4 == 3))
                        chs = wk.tile([P, KC, TP], BF16, tag="chst")
                        nc.vector.tensor_copy(chs[:, :, :n], ps1[:, :, :n])
                        rhs = chs
                    else:
                        hs16 = wk.tile([P, KC, NMAX], BF16, tag="hs16")
                        nc.vector.tensor_scalar_mul(
                            hs16[:, :, :n], hsum_b[:, :, :n], WSCALE)
                        rhs = hs16

                    # full-width iou: one PSUM bank, single accumulation group
                    psit = pp.tile([P, KC, 3, TP], F32, tag="psit")
                    for m in range(KC):
                        for g in range(3):
                            for k in range(KC):
                                nc.tensor.matmul(
                                    psit[:, m, g, :n],
                                    iouhf[:, (m * 3 + g) * KC + k, :],
                                    rhs[:, k, :n],
                                    start=(m == 0 and g == 0 and k == 0),
                                    stop=(m == KC - 1 and g == 2 and k == KC - 1))
                    # full-width fh over child cols
                    psft = pp.tile([P, KC, K * TAILN], F32, tag="psft")
                    for m in range(KC):
                        for k in range(KC):
                            nc.tensor.matmul(
                                psft[:, m, :nch],
                                fhf[:, m * KC + k, :],
                                hch[:, k, :nch],
                                start=(m == 0 and k == 0),
                                stop=(m == KC - 1 and k == KC - 1))

                    IW = 1.0 / WSCALE
                    tmp = wk.tile([P, KC, 3, max(TN, 1)], F32, tag="gtmpt")
                    nc.vector.tensor_scalar_mul(tmp[:, :, :, :n],
                                                psit[:, :, :, :n], IW)
                    nc.vector.tensor_add(
                        tmp[:, :, :, :n], tmp[:, :, :, :n],
                        xi_ful[:, :, 3 * off:3 * (off + n)].rearrange(
                            "p c (t three) -> p c three t", three=3))
                    igt = wk.tile([P, KC, max(TN, 1)], F32, tag="igt")
                    ogt = wk.tile([P, KC, max(TN, 1)], F32, tag="ogt")
                    ugt = wk.tile([P, KC, max(TN, 1)], F32, tag="ugt")
                    nc.scalar.activation(igt[:, :, :n], tmp[:, :, 0, :n],
                                         ACT.Sigmoid)
                    nc.scalar.activation(ogt[:, :, :n], tmp[:, :, 1, :n],
                                         ACT.Sigmoid)
                    nc.scalar.activation(ugt[:, :, :n], tmp[:, :, 2, :n],
                                         ACT.Tanh)
                    cnt_ = wk.tile([P, KC, max(TN, 1)], F32, tag="cnt")
                    nc.vector.tensor_mul(cnt_[:, :, :n], igt[:, :, :n],
                                         ugt[:, :, :n])
                    fsb = wk.tile([P, KC, K * TAILN], F32, tag="fsbt")
                    nc.vector.tensor_scalar_mul(fsb[:, :, :nch],
                                                psft[:, :, :nch], 1.0 / WSCALE)
                    xfb = xf_ful[:, :, off:off + n].rearrange(
                        "p c (n one) -> p c n one", one=1).broadcast_to(
                        (P, KC, n, K))
                    nc.vector.tensor_add(
                        fsb[:, :, :nch].rearrange("p c (n k) -> p c n k", k=K),
                        fsb[:, :, :nch].rearrange("p c (n k) -> p c n k", k=K),
                        xfb)
                    nc.scalar.activation(fsb[:, :, :nch], fsb[:, :, :nch],
                                         ACT.Sigmoid)
                    nc.vector.tensor_mul(fsb[:, :, :nch], fsb[:, :, :nch],
                                         ccg[:, :, :nch])
                    fct = wk.tile([P, KC, max(TN, 1)], F32, tag="fct")
                    nc.vector.tensor_reduce(
                        fct[:, :, :n],
                        fsb[:, :, :nch].rearrange("p c (n k) -> p c n k", k=K),
                        axis=mybir.AxisListType.X, op=mybir.AluOpType.add)
                    nc.vector.tensor_add(cnt_[:, :, :n], cnt_[:, :, :n],
                                         fct[:, :, :n])
                    nc.vector.tensor_copy(c_ful[:, :, TC + off:TC + off + n],
                                          cnt_[:, :, :n])
                    tct = wk.tile([P, KC, max(TN, 1)], F32, tag="tctt")
                    nc.scalar.activation(tct[:, :, :n], cnt_[:, :, :n],
                                         ACT.Tanh)
                    nc.vector.tensor_mul(h_t[:, :, off:off + n],
                                         ogt[:, :, :n], tct[:, :, :n])
                    nc.vector.tensor_copy(h_bf[:, :, p0:p0 + n],
                                          h_t[:, :, off:off + n])

            nc.scalar.dma_start(d_hout.ap(), h_sl[:, :N])
            if TN:
                nc.scalar.dma_start(d_tailh.ap(), h_t[:, :, :TN])

    in_maps = []
    for c in range(NCORES):
        in_maps.append({
            "wsk": wsk_h[c], "wst": wst_h, "masks": mask_h,
            "xt": xt_h, "iouxstat": iouxs_h[c],
            "fxstat": fxs_h[c], "fhstat": fhs_h[c],
            "iouh_full": iouhf_h[c], "fh_full": fhf_h,
            "b_xi": bxi_h[c], "b_iou": biou_h[c], "b_xf": bxf_h[c],
            "b_fh": bfh_h[c],
        })
    _split_multi_waits(nc)
    meta = dict(col_of=col_of, tail_nodes=tail_nodes, N=N, MEM=MEM, TN=TN)
    return nc, in_maps, meta


def _assemble(meta, results):
    col_of, tail_nodes = meta["col_of"], meta["tail_nodes"]
    N, MEM, TN = meta["N"], meta["MEM"], meta["TN"]
    hT = np.concatenate([results[c]["hout"] for c in range(NCORES)], 0)
    out = np.empty((N, MEM), np.float32)
    for node in range(N):
        out[node] = hT[:, col_of[node]]
    if TN:
        th = results[0]["tailh"]  # [P, KC, TN]
        for i, nd in enumerate(tail_nodes):
            out[nd] = th[:, :, i].T.reshape(MEM)
    return out


def kernel(**inputs):
    nc, in_maps, meta = _build(inputs)
    kernel._nc = nc
    kernel._in_maps = in_maps
    res = run_bass_kernel_spmd(nc, in_maps, list(range(NCORES)))
    return _assemble(meta, [res.results[c] for c in range(NCORES)])


# revision 12
# speedup vs baseline: 1.2445x; 1.1296x over previous
"""ChildSumTreeLSTM with relation transforms on 8 Trainium2 NeuronCores.

Layout: everything transposed (features on SBUF partitions, tree nodes on the
free dim), node columns in topological-wave order (heap order). The h/c state
is row-sharded (each core owns a 128-row feature slice for every node), and
the relation matrices are k-chunk-sharded: core c holds the 128-column
contraction chunk c of EVERY relation matrix (fp8, scaled by 16), so the
per-wave relation matmul needs only the core's own h rows:
  local hsum (strided reduce over child cols of the core's own h slice) ->
  per-rel masked hsum copies -> 8 matmuls per rel accumulate a full-height
  partial ch_sum in one PSUM group -> bf16 AllReduce sums the k-chunk
  partials -> row-sharded iou/f gates -> AllGather publishes the new h
  columns (bf16, full height) for the f-gate matmuls and child gathers of
  later waves. The publish AllGather is OFF the critical path: the next
  wave's relation matmul needs only local data, so it overlaps the AG.
The last tiny waves run fully replicated on every core with full-width gate
weights (iouh bf16, merged with the row-sharded copy by per-core block
permutation; fh fp8), needing no collectives; full-width c / xi / xf for the
tail ride piggyback on the publish AllGathers; the host reads the tail
columns from core 0's separate full-width output.
All preload arrays are host-pre-transposed into SBUF layout (contiguous
per-partition DMA lines) and issued on the SP queue; latency-critical
runtime transfers (collective bounces etc.) use the Activation queue so they
are never stuck behind bulk preload triggers.
All per-core differences are input data, so one Bass program runs SPMD on
all 8 cores.
"""

import sys

sys.path.insert(0, "/opt/trn_rl_repo")

import numpy as np
import ml_dtypes

import concourse.bass as bass
import concourse.mybir as mybir
import concourse.tile as tile
from concourse.bass_utils import run_bass_kernel_spmd
from concourse.vector_clock import ScopedClock, VectorClock

BF16 = mybir.dt.bfloat16
F32 = mybir.dt.float32
FP8 = mybir.dt.float8e4
NCORES = 8
P = 128
WSCALE = 16.0
TAILN = 3   # waves with <= this many parents run replicated (no collectives)

# This walrus build rejects >1 sem wait per instruction at the Tile exit
# drain; split the aggregated drain into one drain per proc.
def _split_drain_and_barrier(self, tick_clock, wait_clock):
    gc = tick_clock.global_clock
    n = len(gc)
    nonzero = [i for i in range(n) if gc[i] > 0]
    for j in nonzero:
        vec = VectorClock([gc[i] if i == j else 0 for i in range(n)])
        d = self.nc.sync.drain()
        wait_clock.add_sem_waits(d.ins, ScopedClock({None: vec}))
    if not nonzero:
        d = self.nc.sync.drain()
        wait_clock.add_sem_waits(d.ins, ScopedClock({None: gc.copy()}))
    self.nc.all_engine_barrier()
    assert self.sems is not None
    popped = self.nc._tile_sem_poison_stack.pop()
    assert popped is self._sem_poison
    self.nc.clear_and_free_semaphores(list(self.sems.allocated().values()))
    self.nc.all_engine_barrier()


tile.TileContext._drain_and_barrier = _split_drain_and_barrier


def _split_multi_waits(nc, limit=1):
    """Walrus here allows only one sem wait per instruction; hoist extras
    onto same-engine NOPs inserted right before the instruction."""
    for bb in nc.main_func.blocks:
        new_list = []
        for ins in bb.instructions:
            si = getattr(ins, "sync_info", None)
            if si is not None and si.on_wait and len(si.on_wait) > limit:
                waits = list(si.on_wait)
                for w in waits[:-limit]:
                    nop = mybir.InstNoOp(
                        name=nc.get_next_instruction_name(),
                        sync_info=mybir.SyncInfo(on_wait=[w], on_update=[]),
                        bass_nofuse=True,
                        engine=ins.engine,
                    )
                    nc.register_instruction(nop, overwrite=True)
                    new_list.append(nop)
                si.on_wait = waits[-limit:]
            new_list.append(ins)
        bb.instructions[:] = new_list


def _bf16(a):
    return np.ascontiguousarray(a.astype(ml_dtypes.bfloat16))


def _fp8(a):
    return np.ascontiguousarray(a.astype(ml_dtypes.float8_e4m3))


def _blocksT(mat):
    """[M, K] -> [M/128 * K/128, 128, 128] of transposed blocks, k-major order
    grouped as [m, k] -> index m*KC + k, each block = mat[mb, kb].T (lhsT)."""
    M, K = mat.shape
    MC, KC = M // P, K // P
    out = np.empty((MC * KC, P, P), mat.dtype)
    for m in range(MC):
        for k in range(KC):
            out[m * KC + k] = mat[m * P:(m + 1) * P, k * P:(k + 1) * P].T
    return out


def _plan(child_idx, rel_ids, Wrel):
    """Host-side planning: waves, column order, rel->core assignment, slots."""
    N, K = child_idx.shape
    eff_children = []
    wave = np.zeros(N, np.int32)
    for i in range(N):
        cs = [int(c) for c in child_idx[i] if 0 <= c < i]
        eff_children.append(cs)
        wave[i] = 1 + max((wave[c] for c in cs), default=-1)
    nwaves = int(wave.max()) + 1
    # column order: by (wave, descending node) -> for the reference heap tree
    # this is exactly heap order, keeping children of consecutive parents
    # contiguous.
    order = sorted(range(N), key=lambda i: (wave[i], -i))
    col_of = np.empty(N, np.int64)
    for j, node in enumerate(order):
        col_of[node] = j
    waves = []  # list of (p0, p1) col ranges
    j = 0
    for w in range(nwaves):
        cnt = int((wave == w).sum())
        waves.append((j, j + cnt))
        j += cnt

    ident = set()
    eye = np.eye(Wrel.shape[1], dtype=Wrel.dtype)
    for r in set(int(rel_ids[i]) for i in range(N)):
        if np.array_equal(Wrel[r], eye):
            ident.add(r)

    # first tail wave: every wave from tail_w0 on is tiny and runs replicated
    tail_w0 = nwaves
    while tail_w0 - 1 >= 1 and waves[tail_w0 - 1][1] - waves[tail_w0 - 1][0] <= TAILN:
        tail_w0 -= 1

    # per cooperative wave (1..tail_w0-1): rels present; identity-only waves
    # skip the relation matmul entirely
    wave_rels = []
    for w in range(1, tail_w0):
        p0, p1 = waves[w]
        rels_all = set(int(rel_ids[order[j]]) for j in range(p0, p1))
        if rels_all <= ident:
            wave_rels.append([])
        else:
            wave_rels.append(sorted(rels_all))

    # distinct coop rels ordered by first-use wave (for DMA priority)
    rel_order = []
    for rels in wave_rels:
        for r in rels:
            if r not in rel_order:
                rel_order.append(r)

    # tail waves: replicated slots, one per non-identity rel per wave
    tail_slots = []
    for w in range(tail_w0, nwaves):
        p0, p1 = waves[w]
        rels_all = set(int(rel_ids[order[j]]) for j in range(p0, p1))
        if rels_all <= ident:
            tail_slots.append([])
        else:
            tail_slots.append(sorted(rels_all))

    # cols in cooperative waves whose full-width c a tail wave needs
    tail_child_cols = sorted(set(
        int(col_of[c]) for w in range(tail_w0, nwaves)
        for j in range(waves[w][0], waves[w][1])
        for c in eff_children[order[j]]
        if wave[c] < tail_w0))

    return dict(order=order, col_of=col_of, waves=waves, wave=wave,
                eff_children=eff_children, ident=ident,
                wave_rels=wave_rels, rel_order=rel_order,
                nwaves=nwaves, tail_w0=tail_w0,
                tail_slots=tail_slots, tail_child_cols=tail_child_cols)


def _build(inputs):
    x = np.asarray(inputs["x"], np.float32)
    Wrel = np.asarray(inputs["Wrel"], np.float32)
    ioux_w = np.asarray(inputs["ioux_w"], np.float32)
    ioux_b = np.asarray(inputs["ioux_b"], np.float32)
    iouh_w = np.asarray(inputs["iouh_w"], np.float32)
    iouh_b = np.asarray(inputs["iouh_b"], np.float32)
    fx_w = np.asarray(inputs["fx_w"], np.float32)
    fx_b = np.asarray(inputs["fx_b"], np.float32)
    fh_w = np.asarray(inputs["fh_w"], np.float32)
    fh_b = np.asarray(inputs["fh_b"], np.float32)
    child_idx = np.asarray(inputs["child_idx"], np.int32)
    rel_ids = np.asarray(inputs["rel_ids"], np.int32)

    N, IN_DIM = x.shape
    MEM = fh_w.shape[0]
    KC = MEM // P           # 8 feature chunks
    KX = IN_DIM // P        # 8 input chunks
    K = child_idx.shape[1]  # max children (4)
    NPAD = N + K + 4
    assert KC == NCORES  # publish-AG rank axis doubles as the row-chunk axis

    plan = _plan(child_idx, rel_ids, Wrel)
    order, col_of, waves = plan["order"], plan["col_of"], plan["waves"]
    eff_children = plan["eff_children"]
    wave_rels, rel_order = plan["wave_rels"], plan["rel_order"]
    nwaves = plan["nwaves"]
    tail_w0, tail_slots = plan["tail_w0"], plan["tail_slots"]
    tail_child_cols = plan["tail_child_cols"]

    # Child gather plan: per internal wave, the flattened (parent-major)
    # child column sequence, decomposed into maximal +1-contiguous runs.
    ZCOL = N
    child_col = np.full((N, K), ZCOL, np.int64)
    for i in range(N):
        for kk, c in enumerate(eff_children[i]):
            child_col[i, kk] = col_of[c]
    wave_runs = []
    for w in range(1, nwaves):
        p0, p1 = waves[w]
        seq = []
        for j in range(p0, p1):
            seq.extend(child_col[order[j]])
        runs = []
        i0 = 0
        while i0 < len(seq):
            i1 = i0 + 1
            while i1 < len(seq) and seq[i1] == seq[i1 - 1] + 1:
                i1 += 1
            runs.append((i0, int(seq[i0]), i1 - i0))
            i0 = i1
        wave_runs.append(runs)

    # tail bookkeeping
    TC = len(tail_child_cols)
    tail_nodes = [order[j] for w in range(tail_w0, nwaves)
                  for j in range(waves[w][0], waves[w][1])]
    TN = len(tail_nodes)
    cful_idx = {c: i for i, c in enumerate(tail_child_cols)}
    for i, nd in enumerate(tail_nodes):
        cful_idx[int(col_of[nd])] = TC + i
    pig_cols = []  # per wave 0..tail_w0-1: list of (col_in_wave, cful_pos)
    for w in range(0, tail_w0):
        p0, p1 = waves[w]
        pig_cols.append([(j - p0, cful_idx[j]) for j in range(p0, p1)
                         if j in cful_idx])

    # ---- per-core host data (all pre-transposed into SBUF layout) ----------
    xT = np.ascontiguousarray(x[order].T)
    xt_h = np.zeros((P, KX, N), ml_dtypes.bfloat16)
    for k in range(KX):
        xt_h[:, k, :] = _bf16(xT[k * P:(k + 1) * P])

    NREL = len(rel_order)
    rel_idx = {r: i for i, r in enumerate(rel_order)}
    NMAX = max((waves[w][1] - waves[w][0]) for w in range(1, nwaves)) if nwaves > 1 else 1
    NBIG = max(p1 - p0 for p0, p1 in waves)
    PSN = 128  # psum column pad so each m-chunk slice stays inside one bank
    assert NMAX <= PSN and K * NMAX <= 512
    MC = MEM // P

    # k-chunk relation weights: core c holds lhsT blocks of W_r[:, c*128:...]
    # wsk_h[c][p, ri*MC + m, q] = (W_r * WSCALE)[m*128+q, c*128+p]
    wsk_h = [np.zeros((P, max(NREL, 1) * MC, P), ml_dtypes.float8_e4m3)
             for _ in range(NCORES)]
    for ri, r in enumerate(rel_order):
        Ws = (Wrel[r] * WSCALE).astype(ml_dtypes.float8_e4m3)
        for c in range(NCORES):
            blk = Ws[:, c * P:(c + 1) * P]           # [MEM, P]
            for m in range(MC):
                wsk_h[c][:, ri * MC + m, :] = blk[m * P:(m + 1) * P, :].T
    # per-(wave, rel) column masks, replicated over partitions
    mask_rows = []  # (wave, rel) -> row index
    mrow = {}
    for wi in range(1, tail_w0):
        p0, p1 = waves[wi]
        for r in wave_rels[wi - 1]:
            mrow[(wi, r)] = len(mask_rows)
            row = np.zeros(NMAX, np.float32)
            for t in range(p1 - p0):
                if int(rel_ids[order[p0 + t]]) == r:
                    row[t] = 1.0
            mask_rows.append(row)
    # tail replicated slots: full 64-block matrices + masks
    S_tail = sum(len(rl) for rl in tail_slots)
    wst_h = np.zeros((P, max(S_tail, 1) * MC * KC, P), ml_dtypes.float8_e4m3)
    tail_soff = []
    ts_i = 0
    for twi, rl in enumerate(tail_slots):
        w = tail_w0 + twi
        p0, p1 = waves[w]
        tail_soff.append(ts_i)
        for r in rl:
            blkT = (_blocksT(Wrel[r]) * WSCALE).astype(ml_dtypes.float8_e4m3)
            wst_h[:, ts_i * MC * KC:(ts_i + 1) * MC * KC, :] = \
                blkT.transpose(1, 0, 2)
            row = np.zeros(NMAX, np.float32)
            for t in range(p1 - p0):
                if int(rel_ids[order[p0 + t]]) == r:
                    row[t] = 1.0
            mrow[(w, r)] = len(mask_rows)
            mask_rows.append(row)
            ts_i += 1
    NM = max(len(mask_rows), 1)
    mask_h = np.ascontiguousarray(np.broadcast_to(
        np.stack(mask_rows) if mask_rows else np.zeros((1, NMAX), np.float32),
        (P, NM, NMAX)).astype(ml_dtypes.bfloat16))

    iouxs_h = [np.zeros((P, KX * 3, P), ml_dtypes.bfloat16) for _ in range(NCORES)]
    fxs_h = [np.zeros((P, KX, P), ml_dtypes.bfloat16) for _ in range(NCORES)]
    fhs_h = [np.zeros((P, KC, P), ml_dtypes.bfloat16) for _ in range(NCORES)]
    b_xi = [np.zeros((3, P), np.float32) for _ in range(NCORES)]
    b_iou = [np.zeros((3, P), np.float32) for _ in range(NCORES)]
    b_xf = [np.zeros((P,), np.float32) for _ in range(NCORES)]
    b_fh = [np.zeros((P,), np.float32) for _ in range(NCORES)]
    for c in range(NCORES):
        rows = slice(c * P, (c + 1) * P)
        for g in range(3):
            gr = slice(g * MEM + c * P, g * MEM + (c + 1) * P)
            b_xi[c][g] = ioux_b[gr]
            b_iou[c][g] = iouh_b[gr]
            for k in range(KX):
                iouxs_h[c][:, k * 3 + g, :] = _bf16(
                    ioux_w[gr, k * P:(k + 1) * P].T)
        b_xf[c] = fx_b[rows]
        b_fh[c] = fh_b[rows]
        for k in range(KX):
            fxs_h[c][:, k, :] = _bf16(fx_w[rows, k * P:(k + 1) * P].T)
        for k in range(KC):
            fhs_h[c][:, k, :] = _bf16(fh_w[rows, k * P:(k + 1) * P].T)
    bxi_h = [np.ascontiguousarray(b.T.astype(np.float32)) for b in b_xi]
    biou_h = [np.ascontiguousarray(b.T.astype(np.float32)) for b in b_iou]
    bxf_h = [b.reshape(P, 1).astype(np.float32) for b in b_xf]
    bfh_h = [b.reshape(P, 1).astype(np.float32) for b in b_fh]

    # full-width iouh in bf16, per-core block order: m-slot-major with the
    # core's own row-chunk in slot 0 (serves row-sharded coop iou via slot 0
    # and the replicated tail via all slots; tail rows permuted on cores
    # != 0, whose tail output is unused). SBUF layout [P, (ms*3+g)*KC+k, P].
    iouh_blk = _blocksT(iouh_w)  # [(g*KC+m)*KC+k, P, P]
    iouhf_h = []
    for c in range(NCORES):
        sig = [c] + [m for m in range(KC) if m != c]
        buf = np.zeros((P, KC * 3 * KC, P), ml_dtypes.bfloat16)
        for ms in range(KC):
            for g in range(3):
                for k in range(KC):
                    buf[:, (ms * 3 + g) * KC + k, :] = _bf16(
                        iouh_blk[(g * KC + sig[ms]) * KC + k])
        iouhf_h.append(buf)
    fhf_h = np.ascontiguousarray(
        _fp8(_blocksT(fh_w * WSCALE)).transpose(1, 0, 2))  # [P, KC*KC, P]

    # ---- build program ------------------------------------------------------
    nc = bass.Bass("TRN2", target_bir_lowering=False, debug=False,
                   num_devices=NCORES)
    d_wsk = nc.dram_tensor("wsk", list(wsk_h[0].shape), FP8,
                           kind="ExternalInput")
    d_wst = nc.dram_tensor("wst", list(wst_h.shape), FP8,
                           kind="ExternalInput")
    d_mask = nc.dram_tensor("masks", list(mask_h.shape), BF16,
                            kind="ExternalInput")
    d_xt = nc.dram_tensor("xt", [P, KX, N], BF16, kind="ExternalInput")
    d_iouxs = nc.dram_tensor("iouxstat", [P, KX * 3, P], BF16, kind="ExternalInput")
    d_fxs = nc.dram_tensor("fxstat", [P, KX, P], BF16, kind="ExternalInput")
    d_fhs = nc.dram_tensor("fhstat", [P, KC, P], BF16, kind="ExternalInput")
    d_iouhf = nc.dram_tensor("iouh_full", [P, KC * 3 * KC, P], BF16,
                             kind="ExternalInput")
    d_fhf = nc.dram_tensor("fh_full", [P, KC * KC, P], FP8,
                           kind="ExternalInput")
    d_bxi = nc.dram_tensor("b_xi", [P, 3], F32, kind="ExternalInput")
    d_biou = nc.dram_tensor("b_iou", [P, 3], F32, kind="ExternalInput")
    d_bxf = nc.dram_tensor("b_xf", [P, 1], F32, kind="ExternalInput")
    d_bfh = nc.dram_tensor("b_fh", [P, 1], F32, kind="ExternalInput")
    d_hout = nc.dram_tensor("hout", [P, N], F32, kind="ExternalOutput")
    d_tailh = nc.dram_tensor("tailh", [P, KC, max(TN, 1)], F32,
                             kind="ExternalOutput")

    # rels needed by wave 1 (DMA'd first), then the rest
    r_w1 = len(wave_rels[0]) if wave_rels else 0

    with tile.TileContext(nc, num_cores=NCORES) as tc:
        with (
            tc.tile_pool(name="const", bufs=1) as cpool,
            tc.tile_pool(name="state", bufs=1) as spool,
            tc.tile_pool(name="hmp", bufs=3) as hmp,
            tc.tile_pool(name="work", bufs=1) as wk,
            tc.tile_pool(name="psum", bufs=1, space="PSUM") as pp,
            tc.tile_pool(name="psg", bufs=2, space="PSUM") as pg,
            tc.tile_pool(name="dram", bufs=2, space="DRAM") as dp,
        ):
            # bulk preloads on the SP queue, in priority order; all host
            # arrays are already in SBUF layout (contiguous partition lines)
            xt = cpool.tile([P, KX, N], BF16)
            nc.sync.dma_start(xt[:], d_xt.ap())
            iouxs = cpool.tile([P, KX * 3, P], BF16)
            nc.sync.dma_start(iouxs[:], d_iouxs.ap())
            fxs = cpool.tile([P, KX, P], BF16)
            nc.sync.dma_start(fxs[:], d_fxs.ap())
            bxi = cpool.tile([P, 3], F32)
            nc.sync.dma_start(bxi[:], d_bxi.ap())
            biou = cpool.tile([P, 3], F32)
            nc.sync.dma_start(biou[:], d_biou.ap())
            bxf = cpool.tile([P, 1], F32)
            nc.sync.dma_start(bxf[:], d_bxf.ap())
            bfh = cpool.tile([P, 1], F32)
            nc.sync.dma_start(bfh[:], d_bfh.ap())

            wsk = cpool.tile([P, max(NREL, 1) * MC, P], FP8)
            msk_sb = cpool.tile([P, NM, NMAX], BF16)
            fhs = cpool.tile([P, KC, P], BF16)
            iouhf = cpool.tile([P, KC * 3 * KC, P], BF16)
            fhf = cpool.tile([P, KC * KC, P], FP8)
            wst = cpool.tile([P, max(S_tail, 1) * MC * KC, P], FP8)

            # masks, then wave-1 rels in small chunks (stage 1 starts on
            # the first chunk), then row-sharded gate weights, then the rest
            nc.sync.dma_start(msk_sb[:], d_mask.ap())
            WCH = 6 * MC
            for lo in range(0, r_w1 * MC, WCH):
                hi = min(lo + WCH, r_w1 * MC)
                nc.sync.dma_start(wsk[:, lo:hi, :], d_wsk.ap()[:, lo:hi])
            nc.sync.dma_start(iouhf[:, :3 * KC, :], d_iouhf.ap()[:, :3 * KC])
            nc.sync.dma_start(fhs[:], d_fhs.ap())
            if NREL > r_w1:
                nc.sync.dma_start(wsk[:, r_w1 * MC:, :],
                                  d_wsk.ap()[:, r_w1 * MC:])
            if S_tail:
                nc.sync.dma_start(wst[:], d_wst.ap())
            nc.sync.dma_start(iouhf[:, 3 * KC:, :], d_iouhf.ap()[:, 3 * KC:])
            nc.sync.dma_start(fhf[:], d_fhf.ap())

            # tiny warm-up AllGather: absorbs launch skew and the
            # first-collective setup cost while the preload streams
            wu = wk.tile([P, 1], BF16, tag="wu")
            nc.vector.memset(wu[:], 0.0)
            wu_in = dp.tile([P, 1], BF16, tag="wuin")
            nc.scalar.dma_start(wu_in[:], wu[:])
            wu_out = dp.tile([NCORES, P, 1], BF16, tag="wuout",
                             addr_space="Shared")
            nc.gpsimd.collective_compute(
                "AllGather", mybir.AluOpType.bypass,
                ins=[wu_in.opt()], outs=[wu_out.opt()],
                replica_groups=[list(range(NCORES))])

            # state
            h_bf = spool.tile([P, KC, NPAD], BF16)
            nc.vector.memset(h_bf[:], 0.0)
            c_sl = spool.tile([P, NPAD], F32)
            nc.vector.memset(c_sl[:], 0.0)
            h_sl = spool.tile([P, NPAD], F32)
            nc.vector.memset(h_sl[:], 0.0)
            xi_f = spool.tile([P, 3, N], F32)
            xf_f = spool.tile([P, N], F32)
            c_ful = spool.tile([P, KC, max(TC + TN, 1)], BF16)
            xi_ful = spool.tile([P, KC, 3 * max(TN, 1)], BF16)
            xf_ful = spool.tile([P, KC, max(TN, 1)], BF16)

            ACT = mybir.ActivationFunctionType

            # ---- precompute xi/xf (row-sharded: this core's 128 rows) ------
            with nc.named_scope("xi_pre", notify=True):
                CCH = PSN
                for cc in range(0, N, CCH):
                    ncc = min(CCH, N - cc)
                    ps = pg.tile([P, 3, PSN], F32, tag="ps3")
                    for g in range(3):
                        for k in range(KX):
                            nc.tensor.matmul(
                                ps[:, g, :ncc],
                                iouxs[:, k * 3 + g, :],
                                xt[:, k, cc:cc + ncc],
                                start=(k == 0), stop=(k == KX - 1))
                    for g in range(3):
                        nc.scalar.activation(
                            xi_f[:, g, cc:cc + ncc], ps[:, g, :ncc],
                            ACT.Identity, bias=bxi[:, g:g + 1])
                    psf0 = pg.tile([P, K * NMAX], F32, tag="psf")
                    for k in range(KX):
                        nc.tensor.matmul(
                            psf0[:, :ncc], fxs[:, k, :], xt[:, k, cc:cc + ncc],
                            start=(k == 0), stop=(k == KX - 1))
                    nc.scalar.activation(
                        xf_f[:, cc:cc + ncc], psf0[:, :ncc],
                        ACT.Identity, bias=bxf[:, 0:1])

            def gates(p0, n, iou_ps, fh_src, ccg, nch):
                """Row-sharded gate math for parents at cols [p0, p0+n).
                iou_ps carries WSCALE*(iouh @ ch_sum); None for leaves."""
                NW = NMAX
                sfx = ""
                assert n <= NW
                fc = None
                if fh_src is not None:
                    # f-branch first: independent of the AllReduce, so DVE
                    # and ACT work on it while the collective runs
                    fsb = wk.tile([P, K * NMAX], F32, tag="fsb")
                    xfb = xf_f[:, p0:p0 + n].rearrange(
                        "p (n one) -> p n one", one=1).broadcast_to((P, n, K))
                    nc.vector.tensor_add(
                        fsb[:, :nch].rearrange("p (n k) -> p n k", k=K),
                        fh_src.rearrange("p (n k) -> p n k", k=K), xfb)
                    nc.scalar.activation(fsb[:, :nch], fsb[:, :nch],
                                         ACT.Sigmoid, bias=bfh[:, 0:1])
                    nc.vector.tensor_mul(fsb[:, :nch], fsb[:, :nch],
                                         ccg[:, :nch])
                    fc = wk.tile([P, NMAX], F32, tag="fc")
                    nc.vector.tensor_reduce(
                        fc[:, :n],
                        fsb[:, :nch].rearrange("p (n k) -> p n k", k=K),
                        axis=mybir.AxisListType.X, op=mybir.AluOpType.add)
                if iou_ps is None:
                    gsrc = lambda g: xi_f[:, g, p0:p0 + n]
                else:
                    tmp = wk.tile([P, 3, NW], F32, tag="gtmp" + sfx)
                    nc.vector.tensor_scalar_mul(tmp[:, :, :n], iou_ps,
                                                1.0 / WSCALE)
                    nc.vector.tensor_add(tmp[:, :, :n], tmp[:, :, :n],
                                         xi_f[:, :, p0:p0 + n])
                    gsrc = lambda g: tmp[:, g, :n]
                ig = wk.tile([P, NW], F32, tag="ig" + sfx)
                og = wk.tile([P, NW], F32, tag="og" + sfx)
                ug = wk.tile([P, NW], F32, tag="ug" + sfx)
                nc.scalar.activation(ig[:, :n], gsrc(0), ACT.Sigmoid,
                                     bias=biou[:, 0:1])
                nc.scalar.activation(og[:, :n], gsrc(1), ACT.Sigmoid,
                                     bias=biou[:, 1:2])
                nc.scalar.activation(ug[:, :n], gsrc(2), ACT.Tanh,
                                     bias=biou[:, 2:3])
                cn = wk.tile([P, NW], F32, tag="cn" + sfx)
                nc.vector.tensor_mul(cn[:, :n], ig[:, :n], ug[:, :n])
                if fc is not None:
                    nc.vector.tensor_add(cn[:, :n], cn[:, :n], fc[:, :n])
                nc.vector.tensor_copy(c_sl[:, p0:p0 + n], cn[:, :n])
                tc_t = wk.tile([P, NW], F32, tag="tct" + sfx)
                nc.scalar.activation(tc_t[:, :n], cn[:, :n], ACT.Tanh)
                nc.vector.tensor_mul(h_sl[:, p0:p0 + n], og[:, :n], tc_t[:, :n])

            def publish_h(w, p0, n):
                """AllGather this wave's h cols; piggyback full-width c (and,
                on wave 0, bias-folded xi/xf of tail nodes) for the tail.
                Bounce DMAs ride the Activation queue."""
                pig = pig_cols[w]
                npig = len(pig)
                nxi = TN * 4 if w == 0 and TN else 0
                tot = n + npig + nxi
                sfx = f"w{w}"
                hb = wk.tile([P, tot], BF16, tag="hb" + sfx)
                nc.vector.tensor_copy(hb[:, :n], h_sl[:, p0:p0 + n])
                for i, (cw, _) in enumerate(pig):
                    nc.vector.tensor_copy(hb[:, n + i:n + i + 1],
                                          c_sl[:, p0 + cw:p0 + cw + 1])
                if nxi:
                    # [3*TN xi(+biou) | TN xf(+bfh)], t-major xi triples
                    for i, nd in enumerate(tail_nodes):
                        cl = int(col_of[nd])
                        for g in range(3):
                            nc.vector.tensor_scalar_add(
                                hb[:, n + npig + 3 * i + g:n + npig + 3 * i + g + 1],
                                xi_f[:, g, cl:cl + 1], biou[:, g:g + 1])
                        nc.vector.tensor_scalar_add(
                            hb[:, n + npig + 3 * TN + i:n + npig + 3 * TN + i + 1],
                            xf_f[:, cl:cl + 1], bfh[:, 0:1])
                gin = dp.tile([P, tot], BF16, tag="gin" + sfx)
                nc.scalar.dma_start(gin[:], hb[:])
                gout = dp.tile([NCORES, P, tot], BF16, tag="gout" + sfx,
                               addr_space="Shared")
                nc.gpsimd.collective_compute(
                    "AllGather", mybir.AluOpType.bypass,
                    ins=[gin.opt()], outs=[gout.opt()],
                    replica_groups=[list(range(NCORES))])
                nc.scalar.dma_start(
                    h_bf[:, :, p0:p0 + n],
                    gout[:, :, :n].rearrange("k p n -> p k n"))
                if npig:
                    i = 0
                    while i < npig:
                        j = i + 1
                        while (j < npig and pig[j][1] == pig[j - 1][1] + 1
                               and pig[j][0] == pig[j - 1][0] + 1):
                            j += 1
                        _, fp0 = pig[i]
                        nc.scalar.dma_start(
                            c_ful[:, :, fp0:fp0 + (j - i)],
                            gout[:, :, n + i:n + j].rearrange(
                                "k p n -> p k n"))
                        i = j
                if nxi:
                    nc.scalar.dma_start(
                        xi_ful[:, :, :3 * TN],
                        gout[:, :, n + npig:n + npig + 3 * TN].rearrange(
                            "k p x -> p k x"))
                    nc.scalar.dma_start(
                        xf_ful[:, :, :TN],
                        gout[:, :, n + npig + 3 * TN:n + npig + 4 * TN].rearrange(
                            "k p t -> p k t"))

            # ---- wave 0: leaves -------------------------------------------
            with nc.named_scope("leaves", notify=True):
                p0, p1 = waves[0]
                for cc in range(p0, p1, NMAX):
                    gates(cc, min(NMAX, p1 - cc), None, None, None, 0)
                publish_h(0, p0, p1 - p0)

            # ---- cooperative internal waves -------------------------------
            for wi in range(1, tail_w0):
                with nc.named_scope(f"wave{wi}", notify=True):
                    rels = wave_rels[wi - 1]
                    p0, p1 = waves[wi]
                    n = p1 - p0
                    nch = n * K
                    # local hsum from the core's own h rows (h_sl)
                    hcs = wk.tile([P, K * NMAX], F32, tag="hcs")
                    for (dst, src, ln) in wave_runs[wi - 1]:
                        nc.vector.tensor_copy(hcs[:, dst:dst + ln],
                                              h_sl[:, src:src + ln])
                    hsum = wk.tile([P, NMAX], F32, tag="hsum")
                    nc.vector.tensor_reduce(
                        hsum[:, :n],
                        hcs[:, :nch].rearrange("p (n c) -> p n c", c=K),
                        axis=mybir.AxisListType.X, op=mybir.AluOpType.add)

                    if rels:
                        # k-chunk partial ch_sum over all rels, one PSUM group
                        ps1 = pp.tile([P, MC, PSN], F32, tag="ps1")
                        for ri, r in enumerate(rels):
                            hm = hmp.tile([P, NMAX], BF16, tag="hm")
                            nc.vector.tensor_mul(
                                hm[:, :n], hsum[:, :n],
                                msk_sb[:, mrow[(wi, r)], :n])
                            gi = rel_idx[r]
                            for m in range(MC):
                                nc.tensor.matmul(
                                    ps1[:, m, :n],
                                    wsk[:, gi * MC + m, :],
                                    hm[:, :n],
                                    start=(ri == 0 and m % 4 == 0),
                                    stop=(ri == len(rels) - 1 and m % 4 == 3))
                        cb = wk.tile([P, KC, n], BF16, tag="cb" + str(n))
                        nc.vector.tensor_copy(cb[:, :, :n], ps1[:, :, :n])
                        g1in = dp.tile([P, KC, n], BF16, tag="g1in" + str(n))
                        nc.scalar.dma_start(g1in[:], cb[:])
                        g1out = dp.tile([P, KC, n], BF16, tag="g1out" + str(n),
                                        addr_space="Shared")
                        nc.gpsimd.collective_compute(
                            "AllReduce", mybir.AluOpType.add,
                            ins=[g1in.opt()], outs=[g1out.opt()],
                            replica_groups=[list(range(NCORES))])
                        chs_b = wk.tile([P, KC, n], BF16, tag="chsb" + str(n))
                        nc.scalar.dma_start(chs_b[:], g1out[:])
                        rhs = chs_b
                    else:
                        # identity wave: ch_sum == hsum needs full height;
                        # gather from published h and scale by WSCALE
                        hchf = wk.tile([P, KC, K * NMAX], BF16, tag="hch")
                        for (dst, src, ln) in wave_runs[wi - 1]:
                            nc.vector.tensor_copy(hchf[:, :, dst:dst + ln],
                                                  h_bf[:, :, src:src + ln])
                        hsf = wk.tile([P, KC, NMAX], F32, tag="hsumf")
                        nc.vector.tensor_reduce(
                            hsf[:, :, :n],
                            hchf[:, :, :nch].rearrange(
                                "p k (n c) -> p k n c", c=K),
                            axis=mybir.AxisListType.X, op=mybir.AluOpType.add)
                        hs16 = wk.tile([P, KC, NMAX], BF16, tag="hs16")
                        nc.vector.tensor_scalar_mul(
                            hs16[:, :, :n], hsf[:, :, :n], WSCALE)
                        rhs = hs16

                    # gather full-height child h (for fh) and own-row c
                    hch = wk.tile([P, KC, K * NMAX], BF16, tag="hch")
                    ccg = wk.tile([P, K * NMAX], F32, tag="ccg")
                    for (dst, src, ln) in wave_runs[wi - 1]:
                        nc.vector.tensor_copy(hch[:, :, dst:dst + ln],
                                              h_bf[:, :, src:src + ln])
                        nc.vector.tensor_copy(ccg[:, dst:dst + ln],
                                              c_sl[:, src:src + ln])
                    # fh matmuls first: independent of the AllReduce
                    psf = pg.tile([P, K * NMAX], F32, tag="psf")
                    for k in range(KC):
                        nc.tensor.matmul(
                            psf[:, :nch], fhs[:, k, :], hch[:, k, :nch],
                            start=(k == 0), stop=(k == KC - 1))
                    psi = pg.tile([P, 3, PSN], F32, tag="ps3")
                    for g in range(3):
                        for k in range(KC):
                            nc.tensor.matmul(
                                psi[:, g, :n], iouhf[:, g * KC + k, :],
                                rhs[:, k, :n],
                                start=(k == 0), stop=(k == KC - 1))
                    gates(p0, n, psi[:, :, :n], psf[:, :nch], ccg, nch)
                    publish_h(wi, p0, n)

            # ---- replicated tail waves (no collectives) -------------------
            h_t = wk.tile([P, KC, max(TN, 1)], F32, tag="ht")
            t0c = waves[tail_w0][0] if tail_w0 < nwaves else 0
            TP = 16  # psum col pad for tail iou (keeps psit in one bank)
            for twi in range(nwaves - tail_w0):
                w = tail_w0 + twi
                with nc.named_scope(f"tail{w}", notify=True):
                    rl = tail_slots[twi]
                    p0, p1 = waves[w]
                    n = p1 - p0
                    nch = n * K
                    off = p0 - t0c
                    assert n <= TP and nch <= K * TAILN
                    hch = wk.tile([P, KC, K * NMAX], BF16, tag="hch")
                    for (dst, src, ln) in wave_runs[w - 1]:
                        nc.vector.tensor_copy(hch[:, :, dst:dst + ln],
                                              h_bf[:, :, src:src + ln])
                    ccg = wk.tile([P, KC, K * TAILN], F32, tag="ccgt")
                    nc.vector.memset(ccg[:, :, :nch], 0.0)
                    for j in range(p0, p1):
                        for kk, ch in enumerate(eff_children[order[j]]):
                            fp = cful_idx[int(col_of[ch])]
                            d0 = (j - p0) * K + kk
                            nc.vector.tensor_copy(ccg[:, :, d0:d0 + 1],
                                                  c_ful[:, :, fp:fp + 1])
                    hsum_f = wk.tile([P, KC, NMAX], F32, tag="hsumf")
                    nc.vector.tensor_reduce(
                        hsum_f[:, :, :n],
                        hch[:, :, :nch].rearrange("p k (n c) -> p k n c", c=K),
                        axis=mybir.AxisListType.X, op=mybir.AluOpType.add)
                    hsum_b = wk.tile([P, KC, NMAX], BF16, tag="hsumb")
                    nc.vector.tensor_copy(hsum_b[:, :, :n], hsum_f[:, :, :n])

                    if rl:
                        tso = tail_soff[twi]
                        ps1 = pp.tile([P, MC, PSN], F32, tag="ps1")
                        for s, r in enumerate(rl):
                            hm = hmp.tile([P, KC, NMAX], BF16, tag="hmt")
                            nc.vector.tensor_mul(
                                hm[:, :, :n], hsum_b[:, :, :n],
                                msk_sb[:, mrow[(w, r)], :n].rearrange(
                                    "p (one n) -> p one n", one=1
                                ).broadcast_to((P, KC, n)))
                            wof = (tso + s) * MC * KC
                            for m in range(MC):
                                for k in range(KC):
                                    nc.tensor.matmul(
                                        ps1[:, m, :n],
                                        wst[:, wof + m * KC + k, :],
                                        hm[:, k, :n],
                                        start=(s == 0 and k == 0 and m % 4 == 0),
                                        stop=(s == len(rl) - 1 and k == KC - 1
                                              and m % 4 == 3))
                        chs = wk.tile([P, KC, TP], BF16, tag="chst")
                        nc.vector.tensor_copy(chs[:, :, :n], ps1[:, :, :n])
                        rhs = chs
                    else:
                        hs16 = wk.tile([P, KC, NMAX], BF16, tag="hs16")
                        nc.vector.tensor_scalar_mul(
                            hs16[:, :, :n], hsum_b[:, :, :n], WSCALE)
                        rhs = hs16

                    # full-width iou: one PSUM bank, single accumulation group
                    psit = pp.tile([P, KC, 3, TP], F32, tag="psit")
                    for m in range(KC):
                        for g in range(3):
                            for k in range(KC):
                                nc.tensor.matmul(
                                    psit[:, m, g, :n],
                                    iouhf[:, (m * 3 + g) * KC + k, :],
                                    rhs[:, k, :n],
                                    start=(m == 0 and g == 0 and k == 0),
                                    stop=(m == KC - 1 and g == 2 and k == KC - 1))
                    # full-width fh over child cols
                    psft = pp.tile([P, KC, K * TAILN], F32, tag="psft")
                    for m in range(KC):
                        for k in range(KC):
                            nc.tensor.matmul(
                                psft[:, m, :nch],
                                fhf[:, m * KC + k, :],
                                hch[:, k, :nch],
                                start=(m == 0 and k == 0),
                                stop=(m == KC - 1 and k == KC - 1))

                    IW = 1.0 / WSCALE
                    tmp = wk.tile([P, KC, 3, max(TN, 1)], F32, tag="gtmpt")
                    nc.vector.tensor_scalar_mul(tmp[:, :, :, :n],
                                                psit[:, :, :, :n], IW)
                    nc.vector.tensor_add(
                        tmp[:, :, :, :n], tmp[:, :, :, :n],
                        xi_ful[:, :, 3 * off:3 * (off + n)].rearrange(
                            "p c (t three) -> p c three t", three=3))
                    igt = wk.tile([P, KC, max(TN, 1)], F32, tag="igt")
                    ogt = wk.tile([P, KC, max(TN, 1)], F32, tag="ogt")
                    ugt = wk.tile([P, KC, max(TN, 1)], F32, tag="ugt")
                    nc.scalar.activation(igt[:, :, :n], tmp[:, :, 0, :n],
                                         ACT.Sigmoid)
                    nc.scalar.activation(ogt[:, :, :n], tmp[:, :, 1, :n],
                                         ACT.Sigmoid)
                    nc.scalar.activation(ugt[:, :, :n], tmp[:, :, 2, :n],
                                         ACT.Tanh)
                    cnt_ = wk.tile([P, KC, max(TN, 1)], F32, tag="cnt")
                    nc.vector.tensor_mul(cnt_[:, :, :n], igt[:, :, :n],
                                         ugt[:, :, :n])
                    fsb = wk.tile([P, KC, K * TAILN], F32, tag="fsbt")
                    nc.vector.tensor_scalar_mul(fsb[:, :, :nch],
                                                psft[:, :, :nch], 1.0 / WSCALE)
                    xfb = xf_ful[:, :, off:off + n].rearrange(
                        "p c (n one) -> p c n one", one=1).broadcast_to(
                        (P, KC, n, K))
                    nc.vector.tensor_add(
                        fsb[:, :, :nch].rearrange("p c (n k) -> p c n k", k=K),
                        fsb[:, :, :nch].rearrange("p c (n k) -> p c n k", k=K),
                        xfb)
                    nc.scalar.activation(fsb[:, :, :nch], fsb[:, :, :nch],
                                         ACT.Sigmoid)
                    nc.vector.tensor_mul(fsb[:, :, :nch], fsb[:, :, :nch],
                                         ccg[:, :, :nch])
                    fct = wk.tile([P, KC, max(TN, 1)], F32, tag="fct")
                    nc.vector.tensor_reduce(
                        fct[:, :, :n],
                        fsb[:, :, :nch].rearrange("p c (n k) -> p c n k", k=K),
                        axis=mybir.AxisListType.X, op=mybir.AluOpType.add)
                    nc.vector.tensor_add(cnt_[:, :, :n], cnt_[:, :, :n],
                                         fct[:, :, :n])
                    nc.vector.tensor_copy(c_ful[:, :, TC + off:TC + off + n],
                                          cnt_[:, :, :n])
                    tct = wk.tile([P, KC, max(TN, 1)], F32, tag="tctt")
                    nc.scalar.activation(tct[:, :, :n], cnt_[:, :, :n],
                                         ACT.Tanh)
                    nc.vector.tensor_mul(h_t[:, :, off:off + n],
                                         ogt[:, :, :n], tct[:, :, :n])
                    nc.vector.tensor_copy(h_bf[:, :, p0:p0 + n],
                                          h_t[:, :, off:off + n])

            nc.scalar.dma_start(d_hout.ap(), h_sl[:, :N])
            if TN:
                nc.scalar.dma_start(d_tailh.ap(), h_t[:, :, :TN])

    in_maps = []
    for c in range(NCORES):
        in_maps.append({
            "wsk": wsk_h[c], "wst": wst_h, "masks": mask_h,
            "xt": xt_h, "iouxstat": iouxs_h[c],
            "fxstat": fxs_h[c], "fhstat": fhs_h[c],
            "iouh_full": iouhf_h[c], "fh_full": fhf_h,
            "b_xi": bxi_h[c], "b_iou": biou_h[c], "b_xf": bxf_h[c],
            "b_fh": bfh_h[c],
        })
    _split_multi_waits(nc)
    meta = dict(col_of=col_of, tail_nodes=tail_nodes, N=N, MEM=MEM, TN=TN)
    return nc, in_maps, meta


def _assemble(meta, results):
    col_of, tail_nodes = meta["col_of"], meta["tail_nodes"]
    N, MEM, TN = meta["N"], meta["MEM"], meta["TN"]
    hT = np.concatenate([results[c]["hout"] for c in range(NCORES)], 0)
    out = np.empty((N, MEM), np.float32)
    for node in range(N):
        out[node] = hT[:, col_of[node]]
    if TN:
        th = results[0]["tailh"]  # [P, KC, TN]
        for i, nd in enumerate(tail_nodes):
            out[nd] = th[:, :, i].T.reshape(MEM)
    return out


def kernel(**inputs):
    nc, in_maps, meta = _build(inputs)
    kernel._nc = nc
    kernel._in_maps = in_maps
    res = run_bass_kernel_spmd(nc, in_maps, list(range(NCORES)))
    return _assemble(meta, [res.results[c] for c in range(NCORES)])
